# revision 98
# baseline (speedup 1.0000x reference)
"""Fused PVT-style transformer block kernel for Trainium2 (8 NeuronCores).

Sharding: pure data-parallel over batch B=8 -> one batch item per core.
Layout: channel-major ("transposed") activations [C(part), N(free)] throughout;
host pre-transposes x and relative_pos, post-transposes the output.

Per-core pipeline (N=3136=56x56 tokens, C=256, 4 heads x 64, KV=784=28x28,
HID=1024):
  LN1 (PE ones-matmul stats off a bf16 input mirror, rstd via exp(-.5 ln())
  so LN shares the attention exps' ACT table set, bf16 2x applies into an
  fp8 output; per-tile hook pipelines the SR conv + q projection) ->
  q/k/v/wo/w2 as fp8 DoubleRow matmuls (2 fp8 weights per PE cell = 0.5
  cyc/col; softmax renormalization makes the attention path insensitive to
  fp8 activation noise, while w1 stays bf16 because the gelu-MLP path
  amplifies it), k/v biases eliminated (bk cancels in softmax, bv folds into
  bo) -> flash attention per (head-pair, q-tile): the two heads' K=64 score
  matmuls issue back-to-back into different PSUM banks so they run
  concurrently in PE row-groups 0-63/64-127; joint 2-head exp on ACT;
  p = exp(s)*exp(rp) on DVE (bf16 2x, partial GPSIMD offload); AV with
  ones-row-augmented V giving the softmax denominator for free, software-
  pipelined one tile behind the score matmuls; wo + residual per q-tile
  inside the loop -> LN2 -> conv1x1 (+gelu; bn1 scale/shift folded into the
  depthwise weights, with -c1/a1 halo padding so no separate bn pass) ->
  3x3 depthwise conv, full-window 58x58 halos, split PE (7 ch-tiles, diag
  matmuls) / DVE (tile 0, bf16 accumulator, row-blocks pipelined at lag-1
  inside the w1 loop) -> gelu (evicts to fp8) -> conv1x1 (bn2/pbn folded) +
  residual -> final 3x3 depthwise conv on PE fused into the w2 loop at
  lag-1 row-blocks (residual folded into center tap, exact-f32 center/
  residual in the DVE evict) -> output.
"""

import numpy as np
import ml_dtypes

B, N, C, NH, DH, KV, HID = 8, 3136, 256, 4, 64, 784, 1024
HS = WS = 56
NT = 448            # n-tile (8 rows of 56)
NNT = N // NT       # 7
KT = 112            # kv tile
NKT = KV // KT      # 7
EPS = 1e-5
BF16 = ml_dtypes.bfloat16
WP = WS + 2         # padded spatial extent (58x58) for 3x3 dwconvs

TAPS9 = [(dy, dx) for dy in (-1, 0, 1) for dx in (-1, 0, 1)]


def tap_idx(dy, dx):
    return (dy + 1) * 3 + (dx + 1)


def _build_program(iters=1):
    import concourse.bacc as bacc
    import concourse.mybir as mybir
    import concourse.tile as tile
    from contextlib import ExitStack

    # The act-table-load pass assigns each activation the FIRST table set
    # that contains its function. By default Ln and Exp resolve to two
    # different sets, forcing a ~1.3us table reload per LN tile. Hiding exp
    # and ln from the earlier single-function sets makes both resolve to
    # natural_log_exp_and_others (ln+exp+square+identity+copy), so
    # LN+attention form one table region and the gelu MLP the only switch.
    # Set POSITIONS are untouched: act_func_set_id stays a valid index into
    # the canonical act_info.json that walrus uses.
    orig_tables = bacc.get_activation_tables

    def _tables_nl_exp_joint(arch):
        t = dict(orig_tables(arch))
        exp_f = mybir.ActivationFunctionType.Exp
        ln_f = mybir.ActivationFunctionType.Ln
        out = {}
        for k, v in t.items():
            if k == "exp_and_others":
                v = v - {exp_f}
            if k == "natural_log":
                v = v - {ln_f}
            out[k] = v
        return out

    bacc.get_activation_tables = _tables_nl_exp_joint

    dt = mybir.dt
    F32, BF, F8 = dt.float32, dt.bfloat16, dt.float8e4
    Alu = mybir.AluOpType
    Act = mybir.ActivationFunctionType
    DR = mybir.MatmulPerfMode.DoubleRow

    nc = bacc.Bacc("TRN2", target_bir_lowering=False, debug=False, num_devices=8)

    def din(name, shape, dtype):
        return nc.dram_tensor(name, shape, dtype, kind="ExternalInput")

    xT_d = din("xT", [C, N], F32)
    xTb_d = din("xTb", [C, N], BF)
    rpT_d = din("rpT", [2, NNT, KT, NKT, 2, NT], BF)
    # fp8 DoubleRow weights: [K%128, 2(k-plane), M] — two fp8 weights per PE
    # cell double the effective contraction per pass
    wqT_d = din("wqT", [128, 2, C], F8)
    wkT_d = din("wkT", [128, 2, C], F8)
    wvT_d = din("wvT", [128, 2, C], F8)
    woT_d = din("woT", [128, 2, C], F8)
    w1T_d = din("w1T", [C, HID], BF)
    w2T_d = din("w2T", [4, 128, 2, C], F8)
    bq_d = din("bq", [C], F32)
    bo_d = din("bo", [C], F32)
    b1_d = din("b1", [HID], F32)
    b2r_d = din("b2r", [C], F32)
    srw_d = din("srw", [C, 4], F32)
    dw9_d = din("dw9", [HID, 9], F32)
    dwb_d = din("dwb", [HID], F32)
    padv_d = din("padv", [HID], F32)
    bk9_d = din("bk9", [C, 9], F32)
    bkb_d = din("bkb", [1, C], BF)
    bkbc_d = din("bkbc", [C], F32)
    eyeb_d = din("eyeb", [128, 128], BF)
    onesr_d = din("onesr", [1, 128], BF)
    fT_d = nc.dram_tensor("fT", [C, N], F32, kind="ExternalOutput")

    with tile.TileContext(nc) as tc, ExitStack() as octx:
        wpool = octx.enter_context(tc.tile_pool(name="weights", bufs=1))
        persist = octx.enter_context(tc.tile_pool(name="persist", bufs=1))

        # input tiles first: LN1's first stats matmul needs xres[*][:, :448];
        # emitting these DMAs before the ~1.7MB of weight loads removes the
        # startup stall (weights aren't needed until q/k/v projections).
        xres = [persist.tile([128, N], F32, tag=f"xres{t}", name=f"xres{t}") for t in range(2)]
        # bf16 mirror of the residual stream: LN stats matmuls read this (a
        # bf16 moving operand streams 1 col/cycle on PE vs 4 for fp32)
        xrb = [persist.tile([128, N], BF, tag=f"xrb{t}", name=f"xrb{t}") for t in range(2)]
        for nt in range(NNT):
            cs = slice(nt * NT, (nt + 1) * NT)
            for t in range(2):
                nc.sync.dma_start(out=xres[t][:, cs], in_=xT_d[t * 128:(t + 1) * 128, cs])
                nc.sync.dma_start(out=xrb[t][:, cs], in_=xTb_d[t * 128:(t + 1) * 128, cs])

        def wload(dram_ap, shape, dtype, tag):
            t = wpool.tile(shape, dtype, tag=tag, name=tag)
            nc.sync.dma_start(out=t[:], in_=dram_ap)
            return t

        wq_sb = wload(wqT_d[:, :, :], [128, 2, C], F8, "wq8")
        wk_sb = wload(wkT_d[:, :, :], [128, 2, C], F8, "wk8")
        wv_sb = wload(wvT_d[:, :, :], [128, 2, C], F8, "wv8")
        wo_sb = wload(woT_d[:, :, :], [128, 2, C], F8, "wo8")
        w1_sb = [wload(w1T_d[k * 128:(k + 1) * 128, :], [128, HID], BF, f"w1{k}") for k in range(2)]
        w2_sb = [wload(w2T_d.ap()[k], [128, 2, C], F8, f"w28_{k}") for k in range(4)]
        eyeb = wload(eyeb_d[:, :], [128, 128], BF, "eyeb")
        onesr = wload(onesr_d[:, :], [1, 128], BF, "onesr")
        srw_sb = [wload(srw_d.ap().rearrange("(t p) k -> p t k", p=128)[:, t, :],
                        [128, 4], F32, f"srw{t}") for t in range(2)]
        dw9_sb = [wload(dw9_d.ap().rearrange("(t p) k -> p t k", p=128)[:, t, :],
                        [128, 9], F32, f"dw9_{t}") for t in range(8)]
        bk9_sb = [wload(bk9_d.ap().rearrange("(t p) k -> p t k", p=128)[:, t, :],
                        [128, 9], F32, f"bk9_{t}") for t in range(2)]

        def vload(dram, n, tag):
            t = wpool.tile([128, n // 128], F32, tag=tag, name=tag)
            nc.sync.dma_start(out=t[:], in_=dram.ap().rearrange("(t p) -> p t", p=128))
            return t

        bq_sb = vload(bq_d, C, "bq")
        bo_sb = vload(bo_d, C, "bo")
        b1_sb = vload(b1_d, HID, "b1")
        b2r_sb = vload(b2r_d, C, "b2r")
        dwb_sb = vload(dwb_d, HID, "dwb")
        padv_sb = vload(padv_d, HID, "padv")
        bkb_row = wload(bkb_d[:, :], [1, C], BF, "bkb")
        bkb_sb = vload(bkbc_d, C, "bkbc")
        onesn = wpool.tile([1, NT], BF, tag="onesn")
        nc.vector.memset(onesn[:], 1.0)
        onescol = wpool.tile([128, 1], F32, tag="onescol")
        nc.vector.memset(onescol[:], 1.0)
        onescol_b = wpool.tile([128, 1], BF, tag="onescol_b")
        nc.vector.memset(onescol_b[:], 1.0)
        epscol = wpool.tile([128, 1], F32, tag="epscol")
        nc.vector.memset(epscol[:], EPS)
        ones58 = wpool.tile([128, WP], BF, tag="ones58")
        nc.vector.memset(ones58[:], 1.0)
        # per-HID-tile halo value columns, broadcast to width-58 rows
        prow = []
        for m in range(8):
            t = wpool.tile([128, WP], BF, tag=f"prow{m}", name=f"prow{m}")
            nc.vector.tensor_scalar_mul(t[:], ones58[:], padv_sb[:, m:m + 1])
            prow.append(t)

        # LN1 output: fp8 [p, ct-plane, n] for DoubleRow q/k/v — softmax
        # renormalization makes the attention path insensitive to fp8's ~2%
        # elementwise noise (measured 4e-5 final). LN2 output: bf16 — the
        # MLP path amplifies h-noise through gelu (5e-3 final), so w1 stays
        # a bf16 matmul.
        hb8 = persist.tile([128, 2, N], F8, tag="hb8", name="hb8")
        hbuf = [persist.tile([128, N], BF, tag=f"hbuf{t}", name=f"hbuf{t}") for t in range(2)]

        def body(suffix):
            if not suffix.endswith("i0"):
                for t in range(2):
                    for nt in range(NNT):
                        cs = slice(nt * NT, (nt + 1) * NT)
                        nc.sync.dma_start(out=xres[t][:, cs],
                                          in_=xT_d[t * 128:(t + 1) * 128, cs])
            run_stage1(suffix)
            run_stage2(suffix)

        def layer_norm(suffix, dst, hook=None, act_evict=True):
            """hbuf <- (xres - mean_c) * rsqrt(var_c + eps). hook(nt) emits
            per-tile consumers right after the tile's apply, so downstream
            work pipelines with the remaining tiles' stats. All applies are
            bf16 at DVE 2x; act_evict routes squares + broadcast-evictions
            through ACT (LN1: DVE-bound window) or DVE (LN2: ACT-bound
            window)."""
            with ExitStack() as ctx:
                sqp = ctx.enter_context(tc.tile_pool(name=f"ln_sq{suffix}", bufs=3))
                stp = ctx.enter_context(tc.tile_pool(name=f"ln_st{suffix}", bufs=2, space="PSUM"))
                bcp = ctx.enter_context(tc.tile_pool(name=f"ln_bc{suffix}", bufs=2, space="PSUM"))
                rowp = ctx.enter_context(tc.tile_pool(name=f"ln_row{suffix}", bufs=2))
                tmpp = ctx.enter_context(tc.tile_pool(name=f"ln_tmp{suffix}", bufs=3))
                for nt in range(NNT):
                    cs = slice(nt * NT, (nt + 1) * NT)
                    st0 = stp.tile([1, NT], F32, tag="st0")
                    for ct in range(2):
                        nc.tensor.matmul(st0[:], onescol_b[:], xrb[ct][:, cs],
                                         start=(ct == 0), stop=(ct == 1))
                    st1 = stp.tile([1, NT], F32, tag="st1")
                    for ct in range(2):
                        sq = sqp.tile([128, NT], BF)
                        with nc.allow_low_precision("bf16 LN square"):
                            nc.vector.tensor_mul(sq[:], xrb[ct][:, cs],
                                                 xrb[ct][:, cs])
                        nc.tensor.matmul(st1[:], onescol_b[:], sq[:],
                                         start=(ct == 0), stop=(ct == 1))
                    m2 = rowp.tile([1, NT], F32, tag="m2")
                    nc.scalar.activation(m2[:], st0[:], Act.Square, scale=1.0 / C)
                    var = rowp.tile([1, NT], F32, tag="var")
                    nc.vector.scalar_tensor_tensor(var[:], st1[:], 1.0 / C, m2[:],
                                                   op0=Alu.mult, op1=Alu.subtract)
                    # rstd = exp(-0.5*ln(var+eps)): ln+exp share an ACT table
                    # set (sqrt does not), so LN never forces a table reload
                    # against the attention exps; also frees DVE (no recip).
                    lv = rowp.tile([1, NT], F32, tag="lv")
                    nc.scalar.activation(lv[:], var[:], Act.Ln, bias=epscol[0:1, :])
                    arow = rowp.tile([1, NT], BF, tag="arow")
                    with nc.allow_low_precision("bf16 rstd broadcast row"):
                        nc.scalar.activation(arow[:], lv[:], Act.Exp, scale=-0.5)
                    crow = rowp.tile([1, NT], BF, tag="crow")
                    nc.vector.scalar_tensor_tensor(crow[:], st0[:], -1.0 / C, arow[:],
                                                   op0=Alu.mult, op1=Alu.mult)
                    aps = bcp.tile([128, NT], F32, tag="abc")
                    nc.tensor.matmul(aps[:], onesr[:], arow[:])
                    cps = bcp.tile([128, NT], F32, tag="abc")
                    nc.tensor.matmul(cps[:], onesr[:], crow[:])
                    asb = tmpp.tile([128, NT], BF, tag="asb")
                    csb = tmpp.tile([128, NT], BF, tag="csb")
                    if act_evict:
                        nc.scalar.copy(asb[:], aps[:])
                        nc.scalar.copy(csb[:], cps[:])
                    else:
                        nc.vector.tensor_copy(asb[:], aps[:])
                        nc.vector.tensor_copy(csb[:], cps[:])
                    with nc.allow_low_precision("low-precision LN apply"):
                        for ct in range(2):
                            t0 = tmpp.tile([128, NT], BF, tag="t0")
                            nc.vector.tensor_mul(t0[:], xrb[ct][:, cs], asb[:])
                            nc.vector.tensor_add(dst(ct, cs), t0[:], csb[:])
                    if hook is not None:
                        hook(nt)

        # ================= stage 1: LN1 + attention =================
        def run_stage1(it):
            ctx = ExitStack()
            apool = ctx.enter_context(tc.tile_pool(name="attn_sb", bufs=1))
            cT8 = apool.tile([128, 2, KV], F8, tag="cT8", name="cT8")
            k_sb = [apool.tile([128, KV], BF, tag=f"k{t}", name=f"k{t}") for t in range(2)]
            v_sb = apool.tile([128, NKT * 260], BF, tag="v", name="v_sb")
            q_sb = [apool.tile([128, N], BF, tag=f"q{t}", name=f"q{t}") for t in range(2)]
            oc8 = apool.tile([128, 2, N], F8, tag="oc8", name="oc8")

            with ExitStack() as pctx:
                mmp = pctx.enter_context(tc.tile_pool(name="proj_ps", bufs=2, space="PSUM"))

                def ln1_hook(nt):
                    # SR 2x2/s2 depthwise conv rows + q projection for this
                    # tile: pipelines with the remaining LN1 tiles.
                    cs = slice(nt * NT, (nt + 1) * NT)
                    r = slice(nt * 4, (nt + 1) * 4)
                    with nc.allow_low_precision("fp8 SR conv"):
                        for ct in range(2):
                            h4 = hb8[:, ct, :].rearrange("p (h a w b) -> p h a w b",
                                                         a=2, b=2, h=28, w=28)
                            c3 = cT8[:, ct, :].rearrange("p (h w) -> p h w", w=28)
                            nc.vector.tensor_scalar_mul(c3[:, r, :], h4[:, r, 0, :, 0],
                                                        srw_sb[ct][:, 0:1])
                            for ky, kx in ((0, 1), (1, 0), (1, 1)):
                                ti = ky * 2 + kx
                                nc.vector.scalar_tensor_tensor(
                                    c3[:, r, :], h4[:, r, ky, :, kx], srw_sb[ct][:, ti:ti + 1],
                                    c3[:, r, :], op0=Alu.mult, op1=Alu.add)
                    for mt in range(2):
                        ps = mmp.tile([128, NT], F32, tag="mm")
                        nc.tensor.matmul(ps[:],
                                         wq_sb[:, :, mt * 128:(mt + 1) * 128],
                                         hb8[:, :, cs], perf_mode=DR,
                                         start=True, stop=True)
                        nc.scalar.activation(q_sb[mt][:, cs], ps[:], Act.Identity,
                                             bias=bq_sb[:, mt:mt + 1])

                layer_norm("1" + it, lambda ct, cs: hb8[:, ct, cs],
                           hook=ln1_hook, act_evict=True)

                # k^T = wk @ cT -> [256, 784] bf16 (no bias: bk cancels in softmax)
                for mt in range(2):
                    for n0, nsz in ((0, 448), (448, 336)):
                        ps = mmp.tile([128, NT], F32, tag="mm")
                        nc.tensor.matmul(ps[:, :nsz],
                                         wk_sb[:, :, mt * 128:(mt + 1) * 128],
                                         cT8[:, :, n0:n0 + nsz],
                                         perf_mode=DR, start=True, stop=True)
                        nc.vector.tensor_copy(k_sb[mt][:, n0:n0 + nsz], ps[:, :nsz])

                # v (+ones col per head) -> v_sb [112, 7*260] bf16 (no bias:
                # bv folds into bo)
                for kt in range(NKT):
                    ps = mmp.tile([128, NT], F32, tag="mm")
                    nc.tensor.matmul(ps[0:KT, 0:C], cT8[:, :, kt * KT:(kt + 1) * KT],
                                     wv_sb[:], perf_mode=DR, start=True, stop=True)
                    v4 = v_sb[0:KT, kt * 260:(kt + 1) * 260].rearrange(
                        "p (h e) -> p h e", e=65)
                    nc.vector.tensor_copy(
                        v4[:, :, 0:64],
                        ps[0:KT, 0:C].rearrange("p (h e) -> p h e", e=64))
                    nc.vector.memset(v4[:, :, 64:65], 1.0)

            # flash attention over head-pairs: per (qt, ht) the two heads'
            # K=64 score matmuls go to PE row-groups 0-63 / 64-127 (derived
            # from lhsT base_partition) and different PSUM banks, so they run
            # concurrently. exp covers both heads in one ACT op; rel-pos bias
            # applied as exp(s)*exp(rp) with host-precomputed exp(rp) on DVE.
            with ExitStack() as pctx:
                rpp = pctx.enter_context(tc.tile_pool(name="rp", bufs=3))
                ppp = pctx.enter_context(tc.tile_pool(name="pexp", bufs=2))
                etp = pctx.enter_context(tc.tile_pool(name="et", bufs=3))
                sps = pctx.enter_context(tc.tile_pool(name="spsum", bufs=2, space="PSUM"))
                ops = pctx.enter_context(tc.tile_pool(name="opsum", bufs=2, space="PSUM"))
                rps = pctx.enter_context(tc.tile_pool(name="rpsum", bufs=1, space="PSUM"))
                wop = pctx.enter_context(tc.tile_pool(name="wo_ps", bufs=1, space="PSUM"))
                rsp = pctx.enter_context(tc.tile_pool(name="rsb", bufs=2))

                def wo_qt(qt):
                    # wo projection + residual for this q-tile, emitted as
                    # soon as both head-pairs' AV is done: spreads the DVE
                    # residual adds across the attention phase.
                    cs = slice(qt * NT, (qt + 1) * NT)
                    for mt in range(2):
                        ps = wop.tile([128, NT], F32, name="wops")
                        nc.tensor.matmul(ps[:], wo_sb[:, :, mt * 128:(mt + 1) * 128],
                                         oc8[:, :, cs], perf_mode=DR,
                                         start=True, stop=True)
                        nc.vector.scalar_tensor_tensor(xres[mt][:, cs], ps[:], bo_sb[:, mt:mt + 1],
                                                       xres[mt][:, cs], op0=Alu.add, op1=Alu.add)
                        nc.gpsimd.tensor_copy(xrb[mt][:, cs], xres[mt][:, cs])

                def do_av(qt, ht, p_t):
                    cs = slice(qt * NT, (qt + 1) * NT)
                    for hh in range(2):
                        h = 2 * ht + hh
                        o_ps = ops.tile([65, NT], F32, name="o_ps")
                        for kt in range(NKT):
                            nc.tensor.matmul(
                                o_ps[:],
                                v_sb[0:KT, kt * 260 + h * 65: kt * 260 + (h + 1) * 65],
                                p_t[:, kt, hh, :], start=(kt == 0), stop=(kt == NKT - 1))
                        rrow = rsp.tile([1, NT], BF, tag="rrow", name="rrow")
                        with nc.allow_low_precision("bf16 softmax denom row"):
                            nc.vector.reciprocal(rrow[:], o_ps[64:65, :])
                        rb_ps = rps.tile([64, NT], F32, name="rb_ps")
                        nc.tensor.matmul(rb_ps[:], onesr[0:1, 0:64], rrow[:])
                        rb_sb = rsp.tile([64, NT], F32, tag="rbsb", name="rb_sb")
                        nc.vector.tensor_copy(rb_sb[:], rb_ps[:])
                        with nc.allow_low_precision("fp8 attn output"):
                            nc.vector.tensor_mul(
                                oc8[hh * 64:(hh + 1) * 64, ht, cs],
                                o_ps[0:64, :], rb_sb[:])

                lag = None
                for qt in range(NNT):
                    cs = slice(qt * NT, (qt + 1) * NT)
                    for ht in range(2):
                        rp_t = rpp.tile([KT, NKT, 2, NT], BF, name="rp_t")
                        nc.sync.dma_start(out=rp_t[:], in_=rpT_d.ap()[ht, qt])
                        p_t = ppp.tile([KT, NKT, 2, NT], BF, name="p_t")
                        for kt in range(NKT):
                            s_ps = sps.tile([KT, 1024], F32, name="s_ps")
                            s3 = s_ps[:].rearrange("p (a b) -> p a b", b=512)
                            for hh in range(2):
                                nc.tensor.matmul(
                                    s3[:, hh, 0:NT],
                                    k_sb[ht][hh * 64:(hh + 1) * 64, kt * KT:(kt + 1) * KT],
                                    q_sb[ht][hh * 64:(hh + 1) * 64, cs],
                                    start=True, stop=True)
                            et = etp.tile([KT, 2, NT], BF, name="et")
                            nc.scalar.activation(et[:, :, :], s3[:, :, 0:NT], Act.Exp)
                            eng = nc.gpsimd if kt <= 1 else nc.vector
                            eng.tensor_mul(p_t[:, kt, :, :], et[:, :, :],
                                           rp_t[:, kt, :, :])
                        if lag is not None:
                            do_av(*lag)
                            if lag[1] == 1:
                                wo_qt(lag[0])
                        lag = (qt, ht, p_t)
                do_av(*lag)
                wo_qt(lag[0])

            ctx.close()

        # ================= stage 2: LN2 + conv-MLP + blk dwconv =================
        # dwconv inputs are halo-padded to 58x58 so all 9 taps are always
        # full-window; the MLP dwconv pads with -c1/a1 (so the folded bn1
        # shift c1 sees an effective zero), the blk dwconv pads with zero.

        def run_stage2(it):
            ctx = ExitStack()
            layer_norm("2" + it, lambda ct, cs: hbuf[ct][:, cs], act_evict=True)
            mpool = ctx.enter_context(tc.tile_pool(name="mlp_ps", bufs=3, space="PSUM"))
            dps = ctx.enter_context(tc.tile_pool(name="dw_ps", bufs=2, space="PSUM"))
            upool = ctx.enter_context(tc.tile_pool(name="u", bufs=3))
            accp = ctx.enter_context(tc.tile_pool(name="dwacc", bufs=2))
            digp = ctx.enter_context(tc.tile_pool(name="diag", bufs=2))
            y2p = ctx.enter_context(tc.tile_pool(name="y2", bufs=1))
            y28 = [y2p.tile([128, 2, N], F8, tag=f"y28_{j}", name=f"y28_{j}")
                   for j in range(4)]
            y2 = [y28[m // 2][:, m % 2, :] for m in range(8)]
            x3p = [y2p.tile([128, WP * WP], F32, tag=f"x3p{t}", name=f"x3p{t}")
                   for t in range(2)]
            x3b = [y2p.tile([128, WP * WP], BF, tag=f"x3b{t}", name=f"x3b{t}")
                   for t in range(2)]

            def build_diag(w9_sb):
                diag = []
                for t in range(9):
                    dg = digp.tile([128, 128], BF, tag=f"dg{t}", name=f"dg{t}")
                    nc.vector.tensor_scalar_mul(dg[:], eyeb[:], w9_sb[:, t:t + 1])
                    diag.append(dg)
                return diag

            def dw_pe(src3, diag, bias_col, dst):
                """3x3 depthwise conv of halo-padded bf16 src3 [128,58,58] via
                PE diag matmuls; gelu evict with bias -> dst bf16."""
                for nt in range(NNT):
                    ps = dps.tile([128, NT], F32, name="dwps")
                    r0 = nt * 8
                    for ti, (dy, dx) in enumerate(TAPS9):
                        nc.tensor.matmul(
                            ps[:], diag[ti][:],
                            src3[:, r0 + 1 + dy:r0 + 9 + dy, 1 + dx:57 + dx],
                            start=(ti == 0), stop=(ti == 8))
                    nc.scalar.activation(dst[:, r0 * WS:(r0 + 8) * WS], ps[:], Act.Gelu,
                                         bias=bias_col)

            def dw_elem(src3, w9_sb, bias_col, dst, splits):
                """3x3 depthwise conv on elementwise engines (bf16
                accumulator), row-split per `splits` = [(eng, rlo, rhi)]:
                center-tap init (+bias), 8 fused taps, gelu evicts on ACT."""
                acc = accp.tile([128, N], BF, name="acc")
                a3 = acc[:].rearrange("p (h w) -> p h w", w=WS)
                with nc.allow_low_precision("bf16 dwconv accumulator"):
                    for eng, rlo, rhi in splits:
                        eng.tensor_scalar(a3[:, rlo:rhi, :],
                                          src3[:, rlo + 1:rhi + 1, 1:57], w9_sb[:, 4:5],
                                          bias_col, op0=Alu.mult, op1=Alu.add)
                        for dy, dx in TAPS9:
                            if (dy, dx) == (0, 0):
                                continue
                            t = tap_idx(dy, dx)
                            eng.scalar_tensor_tensor(
                                a3[:, rlo:rhi, :],
                                src3[:, rlo + 1 + dy:rhi + 1 + dy, 1 + dx:57 + dx],
                                w9_sb[:, t:t + 1], a3[:, rlo:rhi, :],
                                op0=Alu.mult, op1=Alu.add)
                for _, rlo, rhi in splits:
                    nc.scalar.activation(dst[:, rlo * WS:rhi * WS],
                                         acc[:, rlo * WS:rhi * WS], Act.Gelu)

            for m in range(8):
                u = upool.tile([128, WP * WP], BF, name="u")
                u3 = u[:].rearrange("p (h w) -> p h w", w=WP)
                pr_c = prow[m][:].rearrange("p (h w) -> p h w", w=1)
                pr_r = prow[m][:].rearrange("p (h w) -> p h w", h=1)
                nc.gpsimd.tensor_copy(u3[:, :, 0:1], pr_c)
                nc.gpsimd.tensor_copy(u3[:, :, 57:58], pr_c)
                nc.gpsimd.tensor_copy(u3[:, 0:1, :], pr_r)
                nc.gpsimd.tensor_copy(u3[:, 57:58, :], pr_r)
                dw_acc = None
                if m == 0:
                    dw_acc = accp.tile([128, N], BF, name="acc")
                    da3 = dw_acc[:].rearrange("p (h w) -> p h w", w=WS)

                def dw_chunk(j):
                    # DVE 3x3 dwconv for one 8-row block: needs u3 rows
                    # j*8..j*8+10 => evicts j and j+1 done (lag-1 behind w1)
                    r0 = j * 8
                    w9 = dw9_sb[m]
                    with nc.allow_low_precision("bf16 dwconv accumulator"):
                        nc.vector.tensor_scalar(
                            da3[:, r0:r0 + 8, :], u3[:, r0 + 1:r0 + 9, 1:57],
                            w9[:, 4:5], dwb_sb[:, m:m + 1], op0=Alu.mult, op1=Alu.add)
                        for dy, dx in TAPS9:
                            if (dy, dx) == (0, 0):
                                continue
                            t = tap_idx(dy, dx)
                            nc.vector.scalar_tensor_tensor(
                                da3[:, r0:r0 + 8, :],
                                u3[:, r0 + 1 + dy:r0 + 9 + dy, 1 + dx:57 + dx],
                                w9[:, t:t + 1], da3[:, r0:r0 + 8, :],
                                op0=Alu.mult, op1=Alu.add)
                    nc.scalar.activation(y2[m][:, r0 * WS:(r0 + 8) * WS],
                                         dw_acc[:, r0 * WS:(r0 + 8) * WS], Act.Gelu)

                for nt in range(NNT):
                    cs = slice(nt * NT, (nt + 1) * NT)
                    ps = mpool.tile([128, NT], F32, tag="mm", name="mmps")
                    for kt in range(2):
                        nc.tensor.matmul(ps[:], w1_sb[kt][:, m * 128:(m + 1) * 128],
                                         hbuf[kt][:, cs], start=(kt == 0), stop=(kt == 1))
                    nc.scalar.activation(u3[:, nt * 8 + 1:(nt + 1) * 8 + 1, 1:57], ps[:],
                                         Act.Gelu, bias=b1_sb[:, m:m + 1])
                    if dw_acc is not None and nt >= 1:
                        dw_chunk(nt - 1)
                if dw_acc is not None:
                    dw_chunk(NNT - 1)
                else:
                    diag = build_diag(dw9_sb[m])
                    dw_pe(u3, diag, dwb_sb[:, m:m + 1], y2[m])

            # w2 (+bn2/pbn folded bias) + residual -> x3p (padded, f32) + bf16
            # copy, with the final blk dwconv fused in at lag-1 row-blocks:
            # blk(j) needs x3 rows j*8..j*8+8 => ready after w2(j+1). ct=0's
            # neighbor taps accumulate in bf16 on DVE, ct=1 runs on PE via
            # diag matmuls + bias ones-row matmul; both fuse the exact-fp32
            # center/residual in the evict: f = acc + (1 + w_center) * x3.
            taps8 = [t for t in TAPS9 if t != (0, 0)]
            blkdiag = [build_diag(bk9_sb[0]), build_diag(bk9_sb[1])]
            x3v = [(x3p[ct][:].rearrange("p (h w) -> p h w", w=WP),
                    x3b[ct][:].rearrange("p (h w) -> p h w", w=WP)) for ct in range(2)]
            for ct in range(2):
                for t3 in x3v[ct]:
                    nc.vector.memset(t3[:, :, 0:1], 0.0)
                    nc.vector.memset(t3[:, :, 57:58], 0.0)
                    nc.vector.memset(t3[:, 0:1, :], 0.0)
                    nc.vector.memset(t3[:, 57:58, :], 0.0)

            def blk_nt(nt):
                # both ct tiles on PE (diag matmuls; the tail has PE slack
                # and PSUM f32 accumulation beats the old bf16 DVE taps)
                r0 = nt * 8
                for ct in range(2):
                    xp3, xb3 = x3v[ct]
                    ps = dps.tile([128, NT], F32, name="blkps")
                    nc.tensor.matmul(ps[:], bkb_row[0:1, ct * 128:(ct + 1) * 128],
                                     onesn[:], start=True, stop=False)
                    for ti, (dy, dx) in enumerate(taps8):
                        nc.tensor.matmul(
                            ps[:], blkdiag[ct][tap_idx(dy, dx)][:],
                            xb3[:, r0 + 1 + dy:r0 + 9 + dy, 1 + dx:57 + dx],
                            start=False, stop=(ti == len(taps8) - 1))
                    fo = accp.tile([128, NT], F32, tag="fout", name="fout", bufs=3)
                    f3 = fo[:].rearrange("p (h w) -> p h w", w=WS)
                    nc.vector.scalar_tensor_tensor(
                        f3[:, :, :], xp3[:, r0 + 1:r0 + 9, 1:57], bk9_sb[ct][:, 4:5],
                        ps[:].rearrange("p (h w) -> p h w", w=WS),
                        op0=Alu.mult, op1=Alu.add)
                    nc.sync.dma_start(
                        out=fT_d[ct * 128:(ct + 1) * 128, r0 * WS:(r0 + 8) * WS],
                        in_=fo[:])

            for nt in range(NNT):
                for mt in range(2):
                    xp3, xb3 = x3v[mt]
                    cs = slice(nt * NT, (nt + 1) * NT)
                    ps = mpool.tile([128, NT], F32, tag="mm", name="mmps2")
                    for j in range(4):
                        nc.tensor.matmul(ps[:], w2_sb[j][:, :, mt * 128:(mt + 1) * 128],
                                         y28[j][:, :, cs], perf_mode=DR,
                                         start=(j == 0), stop=(j == 3))
                    nc.vector.scalar_tensor_tensor(
                        xp3[:, nt * 8 + 1:(nt + 1) * 8 + 1, 1:57], ps[:], b2r_sb[:, mt:mt + 1],
                        xres[mt][:, cs], op0=Alu.add, op1=Alu.add)
                    nc.gpsimd.tensor_copy(xb3[:, nt * 8 + 1:(nt + 1) * 8 + 1, 1:57],
                                          xp3[:, nt * 8 + 1:(nt + 1) * 8 + 1, 1:57])
                if nt >= 1:
                    blk_nt(nt - 1)
            blk_nt(NNT - 1)
            ctx.close()

        for it in range(iters):
            body(f"_i{it}")

    nc.compile()
    bacc.get_activation_tables = orig_tables
    return nc


_CACHE = {}


def _get_program():
    if "nc" not in _CACHE:
        _CACHE["nc"] = _build_program()
    return _CACHE["nc"]


def _prep_inputs(inputs):
    f64 = np.float64
    g1 = inputs["ln1_g"].astype(f64); b1ln = inputs["ln1_b"].astype(f64)
    g2 = inputs["ln2_g"].astype(f64); b2ln = inputs["ln2_b"].astype(f64)
    scale = DH ** -0.5

    def bn_ac(g, b, m, v):
        a = np.asarray(g, f64) / np.sqrt(np.asarray(v, f64) + EPS)
        return a, np.asarray(b, f64) - np.asarray(m, f64) * a

    wq = np.asarray(inputs["wq"], f64); wk = np.asarray(inputs["wk"], f64)
    wv = np.asarray(inputs["wv"], f64); wo = np.asarray(inputs["wo"], f64)

    wq_eff = wq * g1[None, :] * scale
    bq_eff = (wq @ b1ln + np.asarray(inputs["bq"], f64)) * scale

    sa, sc = bn_ac(inputs["srbn_g"], inputs["srbn_b"], inputs["srbn_m"], inputs["srbn_v"])
    srw4 = np.asarray(inputs["sr_w"], f64).reshape(C, 4)  # [c, ky*2+kx]
    srw_eff = srw4 * (g1 * sa)[:, None]
    d_const = sa * (b1ln * srw4.sum(1) + np.asarray(inputs["sr_b"], f64)) + sc
    # bk would add a per-query-row constant to the scores -> cancels in
    # softmax, so k gets no bias at all. bv shifts o by bv (softmax weights
    # sum to 1) -> fold wo @ bv into bo.
    bv_eff = wv @ d_const + np.asarray(inputs["bv"], f64)
    bo_eff = np.asarray(inputs["bo"], f64) + wo @ bv_eff

    w1 = np.asarray(inputs["w1"], f64)
    w1_eff = w1 * g2[None, :]
    b1_eff = w1 @ b2ln + np.asarray(inputs["b1"], f64)
    a1_, c1_ = bn_ac(inputs["bn1_g"], inputs["bn1_b"], inputs["bn1_m"], inputs["bn1_v"])

    # fold bn1 (u' = a1*g + c1) into the dwconv weights: with w' = dw + I_c,
    # out = sum_t w'[t]*u'(+d) + dwb = sum_t (w'[t]*a1)*g(+d) + c1*sum_t w'[t]
    # + dwb, provided g is halo-padded with -c1/a1 (so u'_pad = 0).
    dw9p = np.asarray(inputs["dw_w"], f64).reshape(HID, 9).copy()
    dw9p[:, 4] += 1.0  # residual fold
    dw9_eff = dw9p * a1_[:, None]
    dwb_eff = np.asarray(inputs["dw_b"], f64) + c1_ * dw9p.sum(1)
    padv = -c1_ / a1_

    pa, pc = bn_ac(inputs["pbn_g"], inputs["pbn_b"], inputs["pbn_m"], inputs["pbn_v"])
    a2_, c2_ = bn_ac(inputs["bn2_g"], inputs["bn2_b"], inputs["bn2_m"], inputs["bn2_v"])
    w2 = np.asarray(inputs["w2"], f64)
    w2_eff = (w2 * pa[None, :]) * a2_[:, None]
    b2_eff = a2_ * (w2 @ pc + np.asarray(inputs["b2"], f64)) + c2_

    bk9 = np.asarray(inputs["blkdw_w"], f64).reshape(C, 9).copy()
    bk9[:, 4] += 1.0
    bkb = np.asarray(inputs["blkdw_b"], f64)

    bf = lambda a: np.ascontiguousarray(np.asarray(a, np.float32)).astype(BF16)
    f32 = lambda a: np.ascontiguousarray(np.asarray(a, np.float32))
    E4 = ml_dtypes.float8_e4m3
    f8 = lambda a: np.ascontiguousarray(np.asarray(a, np.float32)).astype(E4)

    def dr2(wT):  # [K=256, M] -> DoubleRow [128, 2, M] fp8
        wT = np.asarray(wT)
        return f8(wT.reshape(2, 128, -1).transpose(1, 0, 2))

    def dr8(wT):  # [K=1024, M] -> 4 x DoubleRow [4, 128, 2, M] fp8
        wT = np.asarray(wT)
        return f8(wT.reshape(4, 2, 128, -1).transpose(0, 2, 1, 3))

    # rp[h, n, m] -> [ht, qt, m_in_tile, kt, h_in_pair, n_in_tile]
    rp6 = np.exp(np.asarray(inputs["relative_pos"], np.float64)).reshape(
        2, 2, NNT, NT, NKT, KT).transpose(0, 2, 5, 4, 1, 3)

    shared = {
        "rpT": np.ascontiguousarray(rp6).astype(BF16),
        "wqT": dr2(wq_eff.T), "wkT": dr2(wk.T), "wvT": dr2(wv.T),
        "woT": dr2(wo.T), "w1T": bf(w1_eff.T), "w2T": dr8(w2_eff.T),
        "bq": f32(bq_eff), "bo": f32(bo_eff), "b1": f32(b1_eff),
        "b2r": f32(b2_eff), "srw": f32(srw_eff), "dw9": f32(dw9_eff),
        "dwb": f32(dwb_eff), "padv": f32(padv),
        "bk9": f32(bk9), "bkb": bf(bkb[None, :]), "bkbc": f32(bkb),
        "eyeb": np.eye(128, dtype=np.float32).astype(BF16),
        "onesr": np.ones((1, 128), np.float32).astype(BF16),
    }
    x = np.asarray(inputs["x"], np.float32)
    in_maps = []
    for b in range(B):
        m = dict(shared)
        xt = np.ascontiguousarray(x[b].T)
        m["xT"] = xt
        m["xTb"] = xt.astype(BF16)
        in_maps.append(m)
    return in_maps


def kernel(**inputs):
    from concourse.bass_utils import run_bass_kernel_spmd
    nc = _get_program()
    in_maps = _prep_inputs(inputs)
    res = run_bass_kernel_spmd(nc, in_maps, core_ids=list(range(B)))
    out = np.stack([res.results[b]["fT"].T for b in range(B)], axis=0)
    return np.ascontiguousarray(out, dtype=np.float32)


# revision 103
# speedup vs baseline: 1.1717x; 1.1717x over previous
"""Fused PVT-style transformer block kernel for Trainium2 (8 NeuronCores).

Sharding: pure data-parallel over batch B=8 -> one batch item per core.
Layout: channel-major ("transposed") activations [C(part), N(free)] throughout;
host pre-transposes x and relative_pos, post-transposes the output.

Per-core pipeline (N=3136=56x56 tokens, C=256, 4 heads x 64, KV=784=28x28,
HID=1024):
  LN1 (PE ones-matmul stats off a bf16 input mirror, rstd via exp(-.5 ln())
  so LN shares the attention exps' ACT table set, bf16 2x applies into an
  fp8 output; per-tile hook pipelines the SR conv + q projection) ->
  q/k/v/wo/w2 as fp8 DoubleRow matmuls (2 fp8 weights per PE cell = 0.5
  cyc/col; softmax renormalization makes the attention path insensitive to
  fp8 activation noise, while w1 stays bf16 because the gelu-MLP path
  amplifies it), k/v biases eliminated (bk cancels in softmax, bv folds into
  bo) -> flash attention per (head-pair, q-tile): the two heads' K=64 score
  matmuls issue back-to-back into different PSUM banks so they run
  concurrently in PE row-groups 0-63/64-127; joint 2-head exp on ACT;
  p = exp(s)*exp(rp) on DVE (bf16 2x, partial GPSIMD offload); AV with
  ones-row-augmented V giving the softmax denominator for free, software-
  pipelined one tile behind the score matmuls; wo + residual per q-tile
  inside the loop -> LN2 -> conv1x1 (+gelu; bn1 scale/shift folded into the
  depthwise weights, with -c1/a1 halo padding so no separate bn pass) ->
  3x3 depthwise conv, full-window 58x58 halos, split PE (7 ch-tiles, diag
  matmuls) / DVE (tile 0, bf16 accumulator, row-blocks pipelined at lag-1
  inside the w1 loop) -> gelu (evicts to fp8) -> conv1x1 (bn2/pbn folded) +
  residual -> final 3x3 depthwise conv on PE fused into the w2 loop at
  lag-1 row-blocks (residual folded into center tap, exact-f32 center/
  residual in the DVE evict) -> output.
"""

import numpy as np
import ml_dtypes

B, N, C, NH, DH, KV, HID = 8, 3136, 256, 4, 64, 784, 1024
HS = WS = 56
NT = 448            # n-tile (8 rows of 56)
NNT = N // NT       # 7
KT = 112            # kv tile
NKT = KV // KT      # 7
EPS = 1e-5
BF16 = ml_dtypes.bfloat16
WP = WS + 2         # padded spatial extent (58x58) for 3x3 dwconvs

TAPS9 = [(dy, dx) for dy in (-1, 0, 1) for dx in (-1, 0, 1)]


def tap_idx(dy, dx):
    return (dy + 1) * 3 + (dx + 1)


def _build_program(iters=1):
    import concourse.bacc as bacc
    import concourse.mybir as mybir
    import concourse.tile as tile
    from contextlib import ExitStack

    # The act-table-load pass assigns each activation the FIRST table set
    # that contains its function. By default Ln and Exp resolve to two
    # different sets, forcing a ~1.3us table reload per LN tile. Hiding exp
    # and ln from the earlier single-function sets makes both resolve to
    # natural_log_exp_and_others (ln+exp+square+identity+copy), so
    # LN+attention form one table region and the gelu MLP the only switch.
    # Set POSITIONS are untouched: act_func_set_id stays a valid index into
    # the canonical act_info.json that walrus uses.
    orig_tables = bacc.get_activation_tables

    def _tables_nl_exp_joint(arch):
        t = dict(orig_tables(arch))
        exp_f = mybir.ActivationFunctionType.Exp
        ln_f = mybir.ActivationFunctionType.Ln
        out = {}
        for k, v in t.items():
            if k == "exp_and_others":
                v = v - {exp_f}
            if k == "natural_log":
                v = v - {ln_f}
            out[k] = v
        return out

    bacc.get_activation_tables = _tables_nl_exp_joint

    dt = mybir.dt
    F32, BF, F8 = dt.float32, dt.bfloat16, dt.float8e4
    Alu = mybir.AluOpType
    Act = mybir.ActivationFunctionType
    DR = mybir.MatmulPerfMode.DoubleRow

    nc = bacc.Bacc("TRN2", target_bir_lowering=False, debug=False, num_devices=8)

    def din(name, shape, dtype):
        return nc.dram_tensor(name, shape, dtype, kind="ExternalInput")

    xT_d = din("xT", [C, N], F32)
    xTb_d = din("xTb", [C, N], BF)
    rpT_d = din("rpT", [2, NNT, KT, NKT, 2, NT], BF)
    # fp8 DoubleRow weights: [K%128, 2(k-plane), M] — two fp8 weights per PE
    # cell double the effective contraction per pass
    wqT_d = din("wqT", [128, 2, C], F8)
    wkT_d = din("wkT", [128, 2, C], F8)
    wvT_d = din("wvT", [128, 2, C], F8)
    woT_d = din("woT", [128, 2, C], F8)
    w1T_d = din("w1T", [C, HID], BF)
    w2T_d = din("w2T", [4, 128, 2, C], F8)
    bq_d = din("bq", [C], F32)
    bo_d = din("bo", [C], F32)
    b1_d = din("b1", [HID], F32)
    b2r_d = din("b2r", [C], F32)
    srw_d = din("srw", [C, 4], F32)
    dw9_d = din("dw9", [HID, 9], F32)
    dwb_d = din("dwb", [HID], F32)
    padv_d = din("padv", [HID], F32)
    bk9_d = din("bk9", [C, 9], F32)
    bkb_d = din("bkb", [1, C], BF)
    bkbc_d = din("bkbc", [C], F32)
    eyeb_d = din("eyeb", [128, 128], BF)
    onesr_d = din("onesr", [1, 128], BF)
    fT_d = nc.dram_tensor("fT", [C, N], F32, kind="ExternalOutput")

    with tile.TileContext(nc) as tc, ExitStack() as octx:
        wpool = octx.enter_context(tc.tile_pool(name="weights", bufs=1))
        persist = octx.enter_context(tc.tile_pool(name="persist", bufs=1))

        # input tiles first: LN1's first stats matmul needs xres[*][:, :448];
        # emitting these DMAs before the ~1.7MB of weight loads removes the
        # startup stall (weights aren't needed until q/k/v projections).
        xres = [persist.tile([128, N], F32, tag=f"xres{t}", name=f"xres{t}") for t in range(2)]
        # bf16 mirror of the residual stream: LN stats matmuls read this (a
        # bf16 moving operand streams 1 col/cycle on PE vs 4 for fp32)
        xrb = [persist.tile([128, N], BF, tag=f"xrb{t}", name=f"xrb{t}") for t in range(2)]
        for nt in range(NNT):
            cs = slice(nt * NT, (nt + 1) * NT)
            for t in range(2):
                nc.sync.dma_start(out=xres[t][:, cs], in_=xT_d[t * 128:(t + 1) * 128, cs])
                # mirror loads on the ACT queue so both streams issue in
                # parallel (DGE issue time is serial per queue)
                nc.scalar.dma_start(out=xrb[t][:, cs], in_=xTb_d[t * 128:(t + 1) * 128, cs])

        def wload(dram_ap, shape, dtype, tag):
            t = wpool.tile(shape, dtype, tag=tag, name=tag)
            nc.sync.dma_start(out=t[:], in_=dram_ap)
            return t

        wq_sb = wload(wqT_d[:, :, :], [128, 2, C], F8, "wq8")
        wk_sb = wload(wkT_d[:, :, :], [128, 2, C], F8, "wk8")
        wv_sb = wload(wvT_d[:, :, :], [128, 2, C], F8, "wv8")
        wo_sb = wload(woT_d[:, :, :], [128, 2, C], F8, "wo8")
        w1_sb = [wload(w1T_d[k * 128:(k + 1) * 128, :], [128, HID], BF, f"w1{k}") for k in range(2)]
        w2_sb = [wload(w2T_d.ap()[k], [128, 2, C], F8, f"w28_{k}") for k in range(4)]
        eyeb = wload(eyeb_d[:, :], [128, 128], BF, "eyeb")
        onesr = wload(onesr_d[:, :], [1, 128], BF, "onesr")
        srw_sb = [wload(srw_d.ap().rearrange("(t p) k -> p t k", p=128)[:, t, :],
                        [128, 4], F32, f"srw{t}") for t in range(2)]
        dw9_sb = [wload(dw9_d.ap().rearrange("(t p) k -> p t k", p=128)[:, t, :],
                        [128, 9], F32, f"dw9_{t}") for t in range(8)]
        bk9_sb = [wload(bk9_d.ap().rearrange("(t p) k -> p t k", p=128)[:, t, :],
                        [128, 9], F32, f"bk9_{t}") for t in range(2)]

        def vload(dram, n, tag):
            t = wpool.tile([128, n // 128], F32, tag=tag, name=tag)
            nc.sync.dma_start(out=t[:], in_=dram.ap().rearrange("(t p) -> p t", p=128))
            return t

        bq_sb = vload(bq_d, C, "bq")
        bo_sb = vload(bo_d, C, "bo")
        b1_sb = vload(b1_d, HID, "b1")
        b2r_sb = vload(b2r_d, C, "b2r")
        dwb_sb = vload(dwb_d, HID, "dwb")
        padv_sb = vload(padv_d, HID, "padv")
        bkb_row = wload(bkb_d[:, :], [1, C], BF, "bkb")
        bkb_sb = vload(bkbc_d, C, "bkbc")
        onesn = wpool.tile([1, NT], BF, tag="onesn")
        nc.vector.memset(onesn[:], 1.0)
        onescol = wpool.tile([128, 1], F32, tag="onescol")
        nc.vector.memset(onescol[:], 1.0)
        onescol_b = wpool.tile([128, 1], BF, tag="onescol_b")
        nc.vector.memset(onescol_b[:], 1.0)
        epscol = wpool.tile([128, 1], F32, tag="epscol")
        nc.vector.memset(epscol[:], EPS)
        ones58 = wpool.tile([128, WP], BF, tag="ones58")
        nc.vector.memset(ones58[:], 1.0)
        # per-HID-tile halo value columns, broadcast to width-58 rows
        prow = []
        for m in range(8):
            t = wpool.tile([128, WP], BF, tag=f"prow{m}", name=f"prow{m}")
            nc.vector.tensor_scalar_mul(t[:], ones58[:], padv_sb[:, m:m + 1])
            prow.append(t)

        # LN1 output: fp8 [p, ct-plane, n] for DoubleRow q/k/v — softmax
        # renormalization makes the attention path insensitive to fp8's ~2%
        # elementwise noise (measured 4e-5 final). LN2 output: bf16 — the
        # MLP path amplifies h-noise through gelu (5e-3 final), so w1 stays
        # a bf16 matmul.
        hb8 = persist.tile([128, 2, N], F8, tag="hb8", name="hb8")
        hbuf = [persist.tile([128, N], BF, tag=f"hbuf{t}", name=f"hbuf{t}") for t in range(2)]

        def body(suffix):
            if not suffix.endswith("i0"):
                for t in range(2):
                    for nt in range(NNT):
                        cs = slice(nt * NT, (nt + 1) * NT)
                        nc.sync.dma_start(out=xres[t][:, cs],
                                          in_=xT_d[t * 128:(t + 1) * 128, cs])
            run_stage1(suffix)
            run_stage2(suffix)

        def layer_norm(suffix, dst, hook=None, act_evict=True):
            """hbuf <- (xres - mean_c) * rsqrt(var_c + eps). hook(nt) emits
            per-tile consumers right after the tile's apply, so downstream
            work pipelines with the remaining tiles' stats. All applies are
            bf16 at DVE 2x; act_evict routes squares + broadcast-evictions
            through ACT (LN1: DVE-bound window) or DVE (LN2: ACT-bound
            window)."""
            with ExitStack() as ctx:
                sqp = ctx.enter_context(tc.tile_pool(name=f"ln_sq{suffix}", bufs=3))
                stp = ctx.enter_context(tc.tile_pool(name=f"ln_st{suffix}", bufs=2, space="PSUM"))
                bcp = ctx.enter_context(tc.tile_pool(name=f"ln_bc{suffix}", bufs=2, space="PSUM"))
                rowp = ctx.enter_context(tc.tile_pool(name=f"ln_row{suffix}", bufs=2))
                tmpp = ctx.enter_context(tc.tile_pool(name=f"ln_tmp{suffix}", bufs=3))
                for nt in range(NNT):
                    cs = slice(nt * NT, (nt + 1) * NT)
                    st0 = stp.tile([1, NT], F32, tag="st0")
                    for ct in range(2):
                        nc.tensor.matmul(st0[:], onescol_b[:], xrb[ct][:, cs],
                                         start=(ct == 0), stop=(ct == 1))
                    st1 = stp.tile([1, NT], F32, tag="st1")
                    for ct in range(2):
                        sq = sqp.tile([128, NT], BF)
                        with nc.allow_low_precision("bf16 LN square"):
                            nc.vector.tensor_mul(sq[:], xrb[ct][:, cs],
                                                 xrb[ct][:, cs])
                        nc.tensor.matmul(st1[:], onescol_b[:], sq[:],
                                         start=(ct == 0), stop=(ct == 1))
                    m2 = rowp.tile([1, NT], F32, tag="m2")
                    nc.scalar.activation(m2[:], st0[:], Act.Square, scale=1.0 / C)
                    var = rowp.tile([1, NT], F32, tag="var")
                    nc.vector.scalar_tensor_tensor(var[:], st1[:], 1.0 / C, m2[:],
                                                   op0=Alu.mult, op1=Alu.subtract)
                    # rstd = exp(-0.5*ln(var+eps)): ln+exp share an ACT table
                    # set (sqrt does not), so LN never forces a table reload
                    # against the attention exps; also frees DVE (no recip).
                    lv = rowp.tile([1, NT], F32, tag="lv")
                    nc.scalar.activation(lv[:], var[:], Act.Ln, bias=epscol[0:1, :])
                    arow = rowp.tile([1, NT], BF, tag="arow")
                    with nc.allow_low_precision("bf16 rstd broadcast row"):
                        nc.scalar.activation(arow[:], lv[:], Act.Exp, scale=-0.5)
                    crow = rowp.tile([1, NT], BF, tag="crow")
                    nc.vector.scalar_tensor_tensor(crow[:], st0[:], -1.0 / C, arow[:],
                                                   op0=Alu.mult, op1=Alu.mult)
                    aps = bcp.tile([128, NT], F32, tag="abc")
                    nc.tensor.matmul(aps[:], onesr[:], arow[:])
                    cps = bcp.tile([128, NT], F32, tag="abc")
                    nc.tensor.matmul(cps[:], onesr[:], crow[:])
                    asb = tmpp.tile([128, NT], BF, tag="asb")
                    csb = tmpp.tile([128, NT], BF, tag="csb")
                    if act_evict:
                        nc.scalar.copy(asb[:], aps[:])
                        nc.scalar.copy(csb[:], cps[:])
                    else:
                        nc.vector.tensor_copy(asb[:], aps[:])
                        nc.vector.tensor_copy(csb[:], cps[:])
                    with nc.allow_low_precision("low-precision LN apply"):
                        for ct in range(2):
                            t0 = tmpp.tile([128, NT], BF, tag="t0")
                            nc.vector.tensor_mul(t0[:], xrb[ct][:, cs], asb[:])
                            nc.vector.tensor_add(dst(ct, cs), t0[:], csb[:])
                    if hook is not None:
                        hook(nt)

        # ================= stage 1: LN1 + attention =================
        def run_stage1(it):
            ctx = ExitStack()
            apool = ctx.enter_context(tc.tile_pool(name="attn_sb", bufs=1))
            cT8 = apool.tile([128, 2, KV], F8, tag="cT8", name="cT8")
            k_sb = [apool.tile([128, KV], BF, tag=f"k{t}", name=f"k{t}") for t in range(2)]
            v_sb = apool.tile([128, NKT * 260], BF, tag="v", name="v_sb")
            q_sb = [apool.tile([128, N], BF, tag=f"q{t}", name=f"q{t}") for t in range(2)]
            oc8 = apool.tile([128, 2, N], F8, tag="oc8", name="oc8")

            with ExitStack() as pctx:
                mmp = pctx.enter_context(tc.tile_pool(name="proj_ps", bufs=2, space="PSUM"))

                def ln1_hook(nt):
                    # SR 2x2/s2 depthwise conv rows + q projection for this
                    # tile: pipelines with the remaining LN1 tiles.
                    cs = slice(nt * NT, (nt + 1) * NT)
                    r = slice(nt * 4, (nt + 1) * 4)
                    with nc.allow_low_precision("fp8 SR conv"):
                        for ct in range(2):
                            h4 = hb8[:, ct, :].rearrange("p (h a w b) -> p h a w b",
                                                         a=2, b=2, h=28, w=28)
                            c3 = cT8[:, ct, :].rearrange("p (h w) -> p h w", w=28)
                            nc.vector.tensor_scalar_mul(c3[:, r, :], h4[:, r, 0, :, 0],
                                                        srw_sb[ct][:, 0:1])
                            for ky, kx in ((0, 1), (1, 0), (1, 1)):
                                ti = ky * 2 + kx
                                nc.vector.scalar_tensor_tensor(
                                    c3[:, r, :], h4[:, r, ky, :, kx], srw_sb[ct][:, ti:ti + 1],
                                    c3[:, r, :], op0=Alu.mult, op1=Alu.add)
                    for mt in range(2):
                        ps = mmp.tile([128, NT], F32, tag="mm")
                        nc.tensor.matmul(ps[:],
                                         wq_sb[:, :, mt * 128:(mt + 1) * 128],
                                         hb8[:, :, cs], perf_mode=DR,
                                         start=True, stop=True)
                        nc.scalar.activation(q_sb[mt][:, cs], ps[:], Act.Identity,
                                             bias=bq_sb[:, mt:mt + 1])

                layer_norm("1" + it, lambda ct, cs: hb8[:, ct, cs],
                           hook=ln1_hook, act_evict=True)

                # k^T = wk @ cT -> [256, 784] bf16 (no bias: bk cancels in softmax)
                for mt in range(2):
                    for n0, nsz in ((0, 448), (448, 336)):
                        ps = mmp.tile([128, NT], F32, tag="mm")
                        nc.tensor.matmul(ps[:, :nsz],
                                         wk_sb[:, :, mt * 128:(mt + 1) * 128],
                                         cT8[:, :, n0:n0 + nsz],
                                         perf_mode=DR, start=True, stop=True)
                        nc.vector.tensor_copy(k_sb[mt][:, n0:n0 + nsz], ps[:, :nsz])

                # v (+ones col per head) -> v_sb [112, 7*260] bf16 (no bias:
                # bv folds into bo)
                for kt in range(NKT):
                    ps = mmp.tile([128, NT], F32, tag="mm")
                    nc.tensor.matmul(ps[0:KT, 0:C], cT8[:, :, kt * KT:(kt + 1) * KT],
                                     wv_sb[:], perf_mode=DR, start=True, stop=True)
                    v4 = v_sb[0:KT, kt * 260:(kt + 1) * 260].rearrange(
                        "p (h e) -> p h e", e=65)
                    nc.vector.tensor_copy(
                        v4[:, :, 0:64],
                        ps[0:KT, 0:C].rearrange("p (h e) -> p h e", e=64))
                    nc.vector.memset(v4[:, :, 64:65], 1.0)

            # flash attention over head-pairs: per (qt, ht) the two heads'
            # K=64 score matmuls go to PE row-groups 0-63 / 64-127 (derived
            # from lhsT base_partition) and different PSUM banks, so they run
            # concurrently. exp covers both heads in one ACT op; rel-pos bias
            # applied as exp(s)*exp(rp) with host-precomputed exp(rp) on DVE.
            with ExitStack() as pctx:
                rpp = pctx.enter_context(tc.tile_pool(name="rp", bufs=3))
                ppp = pctx.enter_context(tc.tile_pool(name="pexp", bufs=2))
                etp = pctx.enter_context(tc.tile_pool(name="et", bufs=3))
                sps = pctx.enter_context(tc.tile_pool(name="spsum", bufs=2, space="PSUM"))
                ops = pctx.enter_context(tc.tile_pool(name="opsum", bufs=2, space="PSUM"))
                rps = pctx.enter_context(tc.tile_pool(name="rpsum", bufs=1, space="PSUM"))
                wop = pctx.enter_context(tc.tile_pool(name="wo_ps", bufs=1, space="PSUM"))
                rsp = pctx.enter_context(tc.tile_pool(name="rsb", bufs=2))

                def wo_qt(qt):
                    # wo projection + residual for this q-tile, emitted as
                    # soon as both head-pairs' AV is done: spreads the DVE
                    # residual adds across the attention phase.
                    cs = slice(qt * NT, (qt + 1) * NT)
                    for mt in range(2):
                        ps = wop.tile([128, NT], F32, name="wops")
                        nc.tensor.matmul(ps[:], wo_sb[:, :, mt * 128:(mt + 1) * 128],
                                         oc8[:, :, cs], perf_mode=DR,
                                         start=True, stop=True)
                        nc.vector.scalar_tensor_tensor(xres[mt][:, cs], ps[:], bo_sb[:, mt:mt + 1],
                                                       xres[mt][:, cs], op0=Alu.add, op1=Alu.add)
                        nc.gpsimd.tensor_copy(xrb[mt][:, cs], xres[mt][:, cs])

                def do_av(qt, ht, p_t):
                    cs = slice(qt * NT, (qt + 1) * NT)
                    for hh in range(2):
                        h = 2 * ht + hh
                        o_ps = ops.tile([65, NT], F32, name="o_ps")
                        for kt in range(NKT):
                            nc.tensor.matmul(
                                o_ps[:],
                                v_sb[0:KT, kt * 260 + h * 65: kt * 260 + (h + 1) * 65],
                                p_t[:, kt, hh, :], start=(kt == 0), stop=(kt == NKT - 1))
                        rrow = rsp.tile([1, NT], BF, tag="rrow", name="rrow")
                        with nc.allow_low_precision("bf16 softmax denom row"):
                            nc.vector.reciprocal(rrow[:], o_ps[64:65, :])
                        rb_ps = rps.tile([64, NT], F32, name="rb_ps")
                        nc.tensor.matmul(rb_ps[:], onesr[0:1, 0:64], rrow[:])
                        rb_sb = rsp.tile([64, NT], F32, tag="rbsb", name="rb_sb")
                        nc.vector.tensor_copy(rb_sb[:], rb_ps[:])
                        with nc.allow_low_precision("fp8 attn output"):
                            nc.vector.tensor_mul(
                                oc8[hh * 64:(hh + 1) * 64, ht, cs],
                                o_ps[0:64, :], rb_sb[:])

                lag = None
                for qt in range(NNT):
                    cs = slice(qt * NT, (qt + 1) * NT)
                    for ht in range(2):
                        rp_t = rpp.tile([KT, NKT, 2, NT], BF, name="rp_t")
                        # issue from the Pool DGE queue: SP's queue is busy
                        # serially issuing the input/weight loads at startup,
                        # which otherwise delays the first rp prefetches
                        nc.gpsimd.dma_start(out=rp_t[:], in_=rpT_d.ap()[ht, qt])
                        p_t = ppp.tile([KT, NKT, 2, NT], BF, name="p_t")
                        for kt in range(NKT):
                            s_ps = sps.tile([KT, 1024], F32, name="s_ps")
                            s3 = s_ps[:].rearrange("p (a b) -> p a b", b=512)
                            for hh in range(2):
                                nc.tensor.matmul(
                                    s3[:, hh, 0:NT],
                                    k_sb[ht][hh * 64:(hh + 1) * 64, kt * KT:(kt + 1) * KT],
                                    q_sb[ht][hh * 64:(hh + 1) * 64, cs],
                                    start=True, stop=True)
                            et = etp.tile([KT, 2, NT], BF, name="et")
                            nc.scalar.activation(et[:, :, :], s3[:, :, 0:NT], Act.Exp)
                            eng = nc.gpsimd if kt <= 1 else nc.vector
                            eng.tensor_mul(p_t[:, kt, :, :], et[:, :, :],
                                           rp_t[:, kt, :, :])
                        if lag is not None:
                            do_av(*lag)
                            if lag[1] == 1:
                                wo_qt(lag[0])
                        lag = (qt, ht, p_t)
                do_av(*lag)
                wo_qt(lag[0])

            ctx.close()

        # ================= stage 2: LN2 + conv-MLP + blk dwconv =================
        # dwconv inputs are halo-padded to 58x58 so all 9 taps are always
        # full-window; the MLP dwconv pads with -c1/a1 (so the folded bn1
        # shift c1 sees an effective zero), the blk dwconv pads with zero.

        def run_stage2(it):
            ctx = ExitStack()
            layer_norm("2" + it, lambda ct, cs: hbuf[ct][:, cs], act_evict=True)
            mpool = ctx.enter_context(tc.tile_pool(name="mlp_ps", bufs=3, space="PSUM"))
            dps = ctx.enter_context(tc.tile_pool(name="dw_ps", bufs=2, space="PSUM"))
            upool = ctx.enter_context(tc.tile_pool(name="u", bufs=3))
            accp = ctx.enter_context(tc.tile_pool(name="dwacc", bufs=2))
            digp = ctx.enter_context(tc.tile_pool(name="diag", bufs=2))
            y2p = ctx.enter_context(tc.tile_pool(name="y2", bufs=1))
            y28 = [y2p.tile([128, 2, N], F8, tag=f"y28_{j}", name=f"y28_{j}")
                   for j in range(4)]
            y2 = [y28[m // 2][:, m % 2, :] for m in range(8)]
            x3p = [y2p.tile([128, WP * WP], F32, tag=f"x3p{t}", name=f"x3p{t}")
                   for t in range(2)]
            x3b = [y2p.tile([128, WP * WP], BF, tag=f"x3b{t}", name=f"x3b{t}")
                   for t in range(2)]

            def build_diag(w9_sb):
                diag = []
                for t in range(9):
                    dg = digp.tile([128, 128], BF, tag=f"dg{t}", name=f"dg{t}")
                    nc.vector.tensor_scalar_mul(dg[:], eyeb[:], w9_sb[:, t:t + 1])
                    diag.append(dg)
                return diag

            def dw_pe(src3, diag, bias_col, dst):
                """3x3 depthwise conv of halo-padded bf16 src3 [128,58,58] via
                PE diag matmuls; gelu evict with bias -> dst bf16."""
                for nt in range(NNT):
                    ps = dps.tile([128, NT], F32, name="dwps")
                    r0 = nt * 8
                    for ti, (dy, dx) in enumerate(TAPS9):
                        nc.tensor.matmul(
                            ps[:], diag[ti][:],
                            src3[:, r0 + 1 + dy:r0 + 9 + dy, 1 + dx:57 + dx],
                            start=(ti == 0), stop=(ti == 8))
                    nc.scalar.activation(dst[:, r0 * WS:(r0 + 8) * WS], ps[:], Act.Gelu,
                                         bias=bias_col)

            def dw_elem(src3, w9_sb, bias_col, dst, splits):
                """3x3 depthwise conv on elementwise engines (bf16
                accumulator), row-split per `splits` = [(eng, rlo, rhi)]:
                center-tap init (+bias), 8 fused taps, gelu evicts on ACT."""
                acc = accp.tile([128, N], BF, name="acc")
                a3 = acc[:].rearrange("p (h w) -> p h w", w=WS)
                with nc.allow_low_precision("bf16 dwconv accumulator"):
                    for eng, rlo, rhi in splits:
                        eng.tensor_scalar(a3[:, rlo:rhi, :],
                                          src3[:, rlo + 1:rhi + 1, 1:57], w9_sb[:, 4:5],
                                          bias_col, op0=Alu.mult, op1=Alu.add)
                        for dy, dx in TAPS9:
                            if (dy, dx) == (0, 0):
                                continue
                            t = tap_idx(dy, dx)
                            eng.scalar_tensor_tensor(
                                a3[:, rlo:rhi, :],
                                src3[:, rlo + 1 + dy:rhi + 1 + dy, 1 + dx:57 + dx],
                                w9_sb[:, t:t + 1], a3[:, rlo:rhi, :],
                                op0=Alu.mult, op1=Alu.add)
                for _, rlo, rhi in splits:
                    nc.scalar.activation(dst[:, rlo * WS:rhi * WS],
                                         acc[:, rlo * WS:rhi * WS], Act.Gelu)

            for m in range(8):
                u = upool.tile([128, WP * WP], BF, name="u")
                u3 = u[:].rearrange("p (h w) -> p h w", w=WP)
                pr_c = prow[m][:].rearrange("p (h w) -> p h w", w=1)
                pr_r = prow[m][:].rearrange("p (h w) -> p h w", h=1)
                nc.gpsimd.tensor_copy(u3[:, :, 0:1], pr_c)
                nc.gpsimd.tensor_copy(u3[:, :, 57:58], pr_c)
                nc.gpsimd.tensor_copy(u3[:, 0:1, :], pr_r)
                nc.gpsimd.tensor_copy(u3[:, 57:58, :], pr_r)
                dw_acc = None
                if m == 0:
                    dw_acc = accp.tile([128, N], BF, name="acc")
                    da3 = dw_acc[:].rearrange("p (h w) -> p h w", w=WS)

                def dw_chunk(j):
                    # DVE 3x3 dwconv for one 8-row block: needs u3 rows
                    # j*8..j*8+10 => evicts j and j+1 done (lag-1 behind w1)
                    r0 = j * 8
                    w9 = dw9_sb[m]
                    with nc.allow_low_precision("bf16 dwconv accumulator"):
                        nc.vector.tensor_scalar(
                            da3[:, r0:r0 + 8, :], u3[:, r0 + 1:r0 + 9, 1:57],
                            w9[:, 4:5], dwb_sb[:, m:m + 1], op0=Alu.mult, op1=Alu.add)
                        for dy, dx in TAPS9:
                            if (dy, dx) == (0, 0):
                                continue
                            t = tap_idx(dy, dx)
                            nc.vector.scalar_tensor_tensor(
                                da3[:, r0:r0 + 8, :],
                                u3[:, r0 + 1 + dy:r0 + 9 + dy, 1 + dx:57 + dx],
                                w9[:, t:t + 1], da3[:, r0:r0 + 8, :],
                                op0=Alu.mult, op1=Alu.add)
                    nc.scalar.activation(y2[m][:, r0 * WS:(r0 + 8) * WS],
                                         dw_acc[:, r0 * WS:(r0 + 8) * WS], Act.Gelu)

                for nt in range(NNT):
                    cs = slice(nt * NT, (nt + 1) * NT)
                    ps = mpool.tile([128, NT], F32, tag="mm", name="mmps")
                    for kt in range(2):
                        nc.tensor.matmul(ps[:], w1_sb[kt][:, m * 128:(m + 1) * 128],
                                         hbuf[kt][:, cs], start=(kt == 0), stop=(kt == 1))
                    nc.scalar.activation(u3[:, nt * 8 + 1:(nt + 1) * 8 + 1, 1:57], ps[:],
                                         Act.Gelu, bias=b1_sb[:, m:m + 1])
                    if dw_acc is not None and nt >= 1:
                        dw_chunk(nt - 1)
                if dw_acc is not None:
                    dw_chunk(NNT - 1)
                else:
                    diag = build_diag(dw9_sb[m])
                    dw_pe(u3, diag, dwb_sb[:, m:m + 1], y2[m])

            # w2 (+bn2/pbn folded bias) + residual -> x3p (padded, f32) + bf16
            # copy, with the final blk dwconv fused in at lag-1 row-blocks:
            # blk(j) needs x3 rows j*8..j*8+8 => ready after w2(j+1). ct=0's
            # neighbor taps accumulate in bf16 on DVE, ct=1 runs on PE via
            # diag matmuls + bias ones-row matmul; both fuse the exact-fp32
            # center/residual in the evict: f = acc + (1 + w_center) * x3.
            taps8 = [t for t in TAPS9 if t != (0, 0)]
            blkdiag = [build_diag(bk9_sb[0]), build_diag(bk9_sb[1])]
            x3v = [(x3p[ct][:].rearrange("p (h w) -> p h w", w=WP),
                    x3b[ct][:].rearrange("p (h w) -> p h w", w=WP)) for ct in range(2)]
            for ct in range(2):
                for t3 in x3v[ct]:
                    nc.vector.memset(t3[:, :, 0:1], 0.0)
                    nc.vector.memset(t3[:, :, 57:58], 0.0)
                    nc.vector.memset(t3[:, 0:1, :], 0.0)
                    nc.vector.memset(t3[:, 57:58, :], 0.0)

            def blk_nt(nt):
                # both ct tiles on PE (diag matmuls; the tail has PE slack
                # and PSUM f32 accumulation beats the old bf16 DVE taps)
                r0 = nt * 8
                for ct in range(2):
                    xp3, xb3 = x3v[ct]
                    ps = dps.tile([128, NT], F32, name="blkps")
                    nc.tensor.matmul(ps[:], bkb_row[0:1, ct * 128:(ct + 1) * 128],
                                     onesn[:], start=True, stop=False)
                    for ti, (dy, dx) in enumerate(taps8):
                        nc.tensor.matmul(
                            ps[:], blkdiag[ct][tap_idx(dy, dx)][:],
                            xb3[:, r0 + 1 + dy:r0 + 9 + dy, 1 + dx:57 + dx],
                            start=False, stop=(ti == len(taps8) - 1))
                    fo = accp.tile([128, NT], F32, tag="fout", name="fout", bufs=3)
                    f3 = fo[:].rearrange("p (h w) -> p h w", w=WS)
                    nc.vector.scalar_tensor_tensor(
                        f3[:, :, :], xp3[:, r0 + 1:r0 + 9, 1:57], bk9_sb[ct][:, 4:5],
                        ps[:].rearrange("p (h w) -> p h w", w=WS),
                        op0=Alu.mult, op1=Alu.add)
                    nc.sync.dma_start(
                        out=fT_d[ct * 128:(ct + 1) * 128, r0 * WS:(r0 + 8) * WS],
                        in_=fo[:])

            for nt in range(NNT):
                for mt in range(2):
                    xp3, xb3 = x3v[mt]
                    cs = slice(nt * NT, (nt + 1) * NT)
                    ps = mpool.tile([128, NT], F32, tag="mm", name="mmps2")
                    for j in range(4):
                        nc.tensor.matmul(ps[:], w2_sb[j][:, :, mt * 128:(mt + 1) * 128],
                                         y28[j][:, :, cs], perf_mode=DR,
                                         start=(j == 0), stop=(j == 3))
                    nc.vector.scalar_tensor_tensor(
                        xp3[:, nt * 8 + 1:(nt + 1) * 8 + 1, 1:57], ps[:], b2r_sb[:, mt:mt + 1],
                        xres[mt][:, cs], op0=Alu.add, op1=Alu.add)
                    nc.gpsimd.tensor_copy(xb3[:, nt * 8 + 1:(nt + 1) * 8 + 1, 1:57],
                                          xp3[:, nt * 8 + 1:(nt + 1) * 8 + 1, 1:57])
                if nt >= 1:
                    blk_nt(nt - 1)
            blk_nt(NNT - 1)
            ctx.close()

        for it in range(iters):
            body(f"_i{it}")

    nc.compile()
    bacc.get_activation_tables = orig_tables
    return nc


_CACHE = {}


def _get_program():
    if "nc" not in _CACHE:
        _CACHE["nc"] = _build_program()
    return _CACHE["nc"]


def _prep_inputs(inputs):
    f64 = np.float64
    g1 = inputs["ln1_g"].astype(f64); b1ln = inputs["ln1_b"].astype(f64)
    g2 = inputs["ln2_g"].astype(f64); b2ln = inputs["ln2_b"].astype(f64)
    scale = DH ** -0.5

    def bn_ac(g, b, m, v):
        a = np.asarray(g, f64) / np.sqrt(np.asarray(v, f64) + EPS)
        return a, np.asarray(b, f64) - np.asarray(m, f64) * a

    wq = np.asarray(inputs["wq"], f64); wk = np.asarray(inputs["wk"], f64)
    wv = np.asarray(inputs["wv"], f64); wo = np.asarray(inputs["wo"], f64)

    wq_eff = wq * g1[None, :] * scale
    bq_eff = (wq @ b1ln + np.asarray(inputs["bq"], f64)) * scale

    sa, sc = bn_ac(inputs["srbn_g"], inputs["srbn_b"], inputs["srbn_m"], inputs["srbn_v"])
    srw4 = np.asarray(inputs["sr_w"], f64).reshape(C, 4)  # [c, ky*2+kx]
    srw_eff = srw4 * (g1 * sa)[:, None]
    d_const = sa * (b1ln * srw4.sum(1) + np.asarray(inputs["sr_b"], f64)) + sc
    # bk would add a per-query-row constant to the scores -> cancels in
    # softmax, so k gets no bias at all. bv shifts o by bv (softmax weights
    # sum to 1) -> fold wo @ bv into bo.
    bv_eff = wv @ d_const + np.asarray(inputs["bv"], f64)
    bo_eff = np.asarray(inputs["bo"], f64) + wo @ bv_eff

    w1 = np.asarray(inputs["w1"], f64)
    w1_eff = w1 * g2[None, :]
    b1_eff = w1 @ b2ln + np.asarray(inputs["b1"], f64)
    a1_, c1_ = bn_ac(inputs["bn1_g"], inputs["bn1_b"], inputs["bn1_m"], inputs["bn1_v"])

    # fold bn1 (u' = a1*g + c1) into the dwconv weights: with w' = dw + I_c,
    # out = sum_t w'[t]*u'(+d) + dwb = sum_t (w'[t]*a1)*g(+d) + c1*sum_t w'[t]
    # + dwb, provided g is halo-padded with -c1/a1 (so u'_pad = 0).
    dw9p = np.asarray(inputs["dw_w"], f64).reshape(HID, 9).copy()
    dw9p[:, 4] += 1.0  # residual fold
    dw9_eff = dw9p * a1_[:, None]
    dwb_eff = np.asarray(inputs["dw_b"], f64) + c1_ * dw9p.sum(1)
    padv = -c1_ / a1_

    pa, pc = bn_ac(inputs["pbn_g"], inputs["pbn_b"], inputs["pbn_m"], inputs["pbn_v"])
    a2_, c2_ = bn_ac(inputs["bn2_g"], inputs["bn2_b"], inputs["bn2_m"], inputs["bn2_v"])
    w2 = np.asarray(inputs["w2"], f64)
    w2_eff = (w2 * pa[None, :]) * a2_[:, None]
    b2_eff = a2_ * (w2 @ pc + np.asarray(inputs["b2"], f64)) + c2_

    bk9 = np.asarray(inputs["blkdw_w"], f64).reshape(C, 9).copy()
    bk9[:, 4] += 1.0
    bkb = np.asarray(inputs["blkdw_b"], f64)

    bf = lambda a: np.ascontiguousarray(np.asarray(a, np.float32)).astype(BF16)
    f32 = lambda a: np.ascontiguousarray(np.asarray(a, np.float32))
    E4 = ml_dtypes.float8_e4m3
    f8 = lambda a: np.ascontiguousarray(np.asarray(a, np.float32)).astype(E4)

    def dr2(wT):  # [K=256, M] -> DoubleRow [128, 2, M] fp8
        wT = np.asarray(wT)
        return f8(wT.reshape(2, 128, -1).transpose(1, 0, 2))

    def dr8(wT):  # [K=1024, M] -> 4 x DoubleRow [4, 128, 2, M] fp8
        wT = np.asarray(wT)
        return f8(wT.reshape(4, 2, 128, -1).transpose(0, 2, 1, 3))

    # rp[h, n, m] -> [ht, qt, m_in_tile, kt, h_in_pair, n_in_tile]
    rp6 = np.exp(np.asarray(inputs["relative_pos"], np.float64)).reshape(
        2, 2, NNT, NT, NKT, KT).transpose(0, 2, 5, 4, 1, 3)

    shared = {
        "rpT": np.ascontiguousarray(rp6).astype(BF16),
        "wqT": dr2(wq_eff.T), "wkT": dr2(wk.T), "wvT": dr2(wv.T),
        "woT": dr2(wo.T), "w1T": bf(w1_eff.T), "w2T": dr8(w2_eff.T),
        "bq": f32(bq_eff), "bo": f32(bo_eff), "b1": f32(b1_eff),
        "b2r": f32(b2_eff), "srw": f32(srw_eff), "dw9": f32(dw9_eff),
        "dwb": f32(dwb_eff), "padv": f32(padv),
        "bk9": f32(bk9), "bkb": bf(bkb[None, :]), "bkbc": f32(bkb),
        "eyeb": np.eye(128, dtype=np.float32).astype(BF16),
        "onesr": np.ones((1, 128), np.float32).astype(BF16),
    }
    x = np.asarray(inputs["x"], np.float32)
    in_maps = []
    for b in range(B):
        m = dict(shared)
        xt = np.ascontiguousarray(x[b].T)
        m["xT"] = xt
        m["xTb"] = xt.astype(BF16)
        in_maps.append(m)
    return in_maps


def kernel(**inputs):
    from concourse.bass_utils import run_bass_kernel_spmd
    nc = _get_program()
    in_maps = _prep_inputs(inputs)
    res = run_bass_kernel_spmd(nc, in_maps, core_ids=list(range(B)))
    out = np.stack([res.results[b]["fT"].T for b in range(B)], axis=0)
    return np.ascontiguousarray(out, dtype=np.float32)


# revision 106
# speedup vs baseline: 1.3271x; 1.1326x over previous
"""Fused PVT-style transformer block kernel for Trainium2 (8 NeuronCores).

Sharding: pure data-parallel over batch B=8 -> one batch item per core.
Layout: channel-major ("transposed") activations [C(part), N(free)] throughout;
host pre-transposes x and relative_pos, post-transposes the output.

Per-core pipeline (N=3136=56x56 tokens, C=256, 4 heads x 64, KV=784=28x28,
HID=1024):
  LN1 (PE ones-matmul stats off a bf16 input mirror, rstd via exp(-.5 ln())
  so LN shares the attention exps' ACT table set, bf16 2x applies into an
  fp8 output; per-tile hook pipelines the SR conv + q projection) ->
  q/k/v/wo/w2 as fp8 DoubleRow matmuls (2 fp8 weights per PE cell = 0.5
  cyc/col; softmax renormalization makes the attention path insensitive to
  fp8 activation noise, while w1 stays bf16 because the gelu-MLP path
  amplifies it), k/v biases eliminated (bk cancels in softmax, bv folds into
  bo) -> flash attention per (head-pair, q-tile): the two heads' K=64 score
  matmuls issue back-to-back into different PSUM banks so they run
  concurrently in PE row-groups 0-63/64-127; joint 2-head exp on ACT;
  p = exp(s)*exp(rp) on DVE (bf16 2x, partial GPSIMD offload); AV with
  ones-row-augmented V giving the softmax denominator for free, software-
  pipelined one tile behind the score matmuls; wo + residual per q-tile
  inside the loop -> LN2 -> conv1x1 (+gelu; bn1 scale/shift folded into the
  depthwise weights, with -c1/a1 halo padding so no separate bn pass) ->
  3x3 depthwise conv, full-window 58x58 halos, split PE (7 ch-tiles, diag
  matmuls) / DVE (tile 0, bf16 accumulator, row-blocks pipelined at lag-1
  inside the w1 loop) -> gelu (evicts to fp8) -> conv1x1 (bn2/pbn folded) +
  residual -> final 3x3 depthwise conv on PE fused into the w2 loop at
  lag-1 row-blocks (residual folded into center tap, exact-f32 center/
  residual in the DVE evict) -> output.
"""

import numpy as np
import ml_dtypes

B, N, C, NH, DH, KV, HID = 8, 3136, 256, 4, 64, 784, 1024
HS = WS = 56
NT = 448            # n-tile (8 rows of 56)
NNT = N // NT       # 7
KT = 112            # kv tile
NKT = KV // KT      # 7
EPS = 1e-5
BF16 = ml_dtypes.bfloat16
WP = WS + 2         # padded spatial extent (58x58) for 3x3 dwconvs

TAPS9 = [(dy, dx) for dy in (-1, 0, 1) for dx in (-1, 0, 1)]


def tap_idx(dy, dx):
    return (dy + 1) * 3 + (dx + 1)


def _build_program(iters=1):
    import concourse.bacc as bacc
    import concourse.mybir as mybir
    import concourse.tile as tile
    from contextlib import ExitStack

    # The act-table-load pass assigns each activation the FIRST table set
    # that contains its function. By default Ln and Exp resolve to two
    # different sets, forcing a ~1.3us table reload per LN tile. Hiding exp
    # and ln from the earlier single-function sets makes both resolve to
    # natural_log_exp_and_others (ln+exp+square+identity+copy), so
    # LN+attention form one table region and the gelu MLP the only switch.
    # Set POSITIONS are untouched: act_func_set_id stays a valid index into
    # the canonical act_info.json that walrus uses.
    orig_tables = bacc.get_activation_tables

    def _tables_nl_exp_joint(arch):
        t = dict(orig_tables(arch))
        exp_f = mybir.ActivationFunctionType.Exp
        ln_f = mybir.ActivationFunctionType.Ln
        out = {}
        for k, v in t.items():
            if k == "exp_and_others":
                v = v - {exp_f}
            if k == "natural_log":
                v = v - {ln_f}
            out[k] = v
        return out

    bacc.get_activation_tables = _tables_nl_exp_joint

    dt = mybir.dt
    F32, BF, F8 = dt.float32, dt.bfloat16, dt.float8e4
    Alu = mybir.AluOpType
    Act = mybir.ActivationFunctionType
    DR = mybir.MatmulPerfMode.DoubleRow

    nc = bacc.Bacc("TRN2", target_bir_lowering=False, debug=False, num_devices=8)

    def din(name, shape, dtype):
        return nc.dram_tensor(name, shape, dtype, kind="ExternalInput")

    xT_d = din("xT", [C, N], F32)
    xTb_d = din("xTb", [C, N], BF)
    rpT_d = din("rpT", [2, NNT, KT, NKT, 2, NT], BF)
    # fp8 DoubleRow weights: [K%128, 2(k-plane), M] — two fp8 weights per PE
    # cell double the effective contraction per pass
    wqT_d = din("wqT", [128, 2, C], F8)
    wkT_d = din("wkT", [128, 2, C], F8)
    wvT_d = din("wvT", [128, 2, C], F8)
    woT_d = din("woT", [128, 2, C], F8)
    w1T_d = din("w1T", [C, HID], BF)
    w2T_d = din("w2T", [4, 128, 2, C], F8)
    bq_d = din("bq", [C], F32)
    bo_d = din("bo", [C], F32)
    b1_d = din("b1", [HID], F32)
    b2r_d = din("b2r", [C], F32)
    srw_d = din("srw", [C, 4], F32)
    dw9_d = din("dw9", [HID, 9], F32)
    dwb_d = din("dwb", [HID], F32)
    padv_d = din("padv", [HID], F32)
    bk9_d = din("bk9", [C, 9], F32)
    bkb_d = din("bkb", [1, C], BF)
    bkbc_d = din("bkbc", [C], F32)
    eyeb_d = din("eyeb", [128, 128], BF)
    onesr_d = din("onesr", [1, 128], BF)
    fT_d = nc.dram_tensor("fT", [C, N], F32, kind="ExternalOutput")

    with tile.TileContext(nc) as tc, ExitStack() as octx:
        wpool = octx.enter_context(tc.tile_pool(name="weights", bufs=1))
        persist = octx.enter_context(tc.tile_pool(name="persist", bufs=1))

        # input tiles first: LN1's first stats matmul needs xres[*][:, :448];
        # emitting these DMAs before the ~1.7MB of weight loads removes the
        # startup stall (weights aren't needed until q/k/v projections).
        xres = [persist.tile([128, N], F32, tag=f"xres{t}", name=f"xres{t}") for t in range(2)]
        # bf16 mirror of the residual stream: LN stats matmuls read this (a
        # bf16 moving operand streams 1 col/cycle on PE vs 4 for fp32)
        xrb = [persist.tile([128, N], BF, tag=f"xrb{t}", name=f"xrb{t}") for t in range(2)]
        for nt in range(NNT):
            cs = slice(nt * NT, (nt + 1) * NT)
            for t in range(2):
                nc.sync.dma_start(out=xres[t][:, cs], in_=xT_d[t * 128:(t + 1) * 128, cs])
                # mirror loads on the ACT queue so both streams issue in
                # parallel (DGE issue time is serial per queue)
                nc.scalar.dma_start(out=xrb[t][:, cs], in_=xTb_d[t * 128:(t + 1) * 128, cs])

        def wload(dram_ap, shape, dtype, tag):
            t = wpool.tile(shape, dtype, tag=tag, name=tag)
            nc.sync.dma_start(out=t[:], in_=dram_ap)
            return t

        wq_sb = wload(wqT_d[:, :, :], [128, 2, C], F8, "wq8")
        wk_sb = wload(wkT_d[:, :, :], [128, 2, C], F8, "wk8")
        wv_sb = wload(wvT_d[:, :, :], [128, 2, C], F8, "wv8")
        wo_sb = wload(woT_d[:, :, :], [128, 2, C], F8, "wo8")
        w1_sb = [wload(w1T_d[k * 128:(k + 1) * 128, :], [128, HID], BF, f"w1{k}") for k in range(2)]
        w2_sb = [wload(w2T_d.ap()[k], [128, 2, C], F8, f"w28_{k}") for k in range(4)]
        eyeb = wload(eyeb_d[:, :], [128, 128], BF, "eyeb")
        onesr = wload(onesr_d[:, :], [1, 128], BF, "onesr")
        srw_sb = [wload(srw_d.ap().rearrange("(t p) k -> p t k", p=128)[:, t, :],
                        [128, 4], F32, f"srw{t}") for t in range(2)]
        dw9_sb = [wload(dw9_d.ap().rearrange("(t p) k -> p t k", p=128)[:, t, :],
                        [128, 9], F32, f"dw9_{t}") for t in range(8)]
        bk9_sb = [wload(bk9_d.ap().rearrange("(t p) k -> p t k", p=128)[:, t, :],
                        [128, 9], F32, f"bk9_{t}") for t in range(2)]

        def vload(dram, n, tag):
            t = wpool.tile([128, n // 128], F32, tag=tag, name=tag)
            nc.sync.dma_start(out=t[:], in_=dram.ap().rearrange("(t p) -> p t", p=128))
            return t

        bq_sb = vload(bq_d, C, "bq")
        bo_sb = vload(bo_d, C, "bo")
        b1_sb = vload(b1_d, HID, "b1")
        b2r_sb = vload(b2r_d, C, "b2r")
        dwb_sb = vload(dwb_d, HID, "dwb")
        padv_sb = vload(padv_d, HID, "padv")
        bkb_row = wload(bkb_d[:, :], [1, C], BF, "bkb")
        bkb_sb = vload(bkbc_d, C, "bkbc")
        onesn = wpool.tile([1, NT], BF, tag="onesn")
        nc.vector.memset(onesn[:], 1.0)
        onescol = wpool.tile([128, 1], F32, tag="onescol")
        nc.vector.memset(onescol[:], 1.0)
        onescol_b = wpool.tile([128, 1], BF, tag="onescol_b")
        nc.vector.memset(onescol_b[:], 1.0)
        epscol = wpool.tile([128, 1], F32, tag="epscol")
        nc.vector.memset(epscol[:], EPS)
        ones58 = wpool.tile([128, WP], BF, tag="ones58")
        nc.vector.memset(ones58[:], 1.0)
        # per-HID-tile halo value columns, broadcast to width-58 rows
        prow = []
        for m in range(8):
            t = wpool.tile([128, WP], BF, tag=f"prow{m}", name=f"prow{m}")
            nc.vector.tensor_scalar_mul(t[:], ones58[:], padv_sb[:, m:m + 1])
            prow.append(t)

        # LN1 output: fp8 [p, ct-plane, n] for DoubleRow q/k/v — softmax
        # renormalization makes the attention path insensitive to fp8's ~2%
        # elementwise noise (measured 4e-5 final). LN2 output: bf16 — the
        # MLP path amplifies h-noise through gelu (5e-3 final), so w1 stays
        # a bf16 matmul.
        hb8 = persist.tile([128, 2, N], F8, tag="hb8", name="hb8")
        hbuf = [persist.tile([128, N], BF, tag=f"hbuf{t}", name=f"hbuf{t}") for t in range(2)]

        def body(suffix):
            if not suffix.endswith("i0"):
                for t in range(2):
                    for nt in range(NNT):
                        cs = slice(nt * NT, (nt + 1) * NT)
                        nc.sync.dma_start(out=xres[t][:, cs],
                                          in_=xT_d[t * 128:(t + 1) * 128, cs])
            run_stage1(suffix)
            run_stage2(suffix)

        def layer_norm(suffix, dst, hook=None, act_evict=True):
            """hbuf <- (xres - mean_c) * rsqrt(var_c + eps). hook(nt) emits
            per-tile consumers right after the tile's apply, so downstream
            work pipelines with the remaining tiles' stats. All applies are
            bf16 at DVE 2x; act_evict routes squares + broadcast-evictions
            through ACT (LN1: DVE-bound window) or DVE (LN2: ACT-bound
            window)."""
            with ExitStack() as ctx:
                sqp = ctx.enter_context(tc.tile_pool(name=f"ln_sq{suffix}", bufs=3))
                stp = ctx.enter_context(tc.tile_pool(name=f"ln_st{suffix}", bufs=2, space="PSUM"))
                bcp = ctx.enter_context(tc.tile_pool(name=f"ln_bc{suffix}", bufs=2, space="PSUM"))
                rowp = ctx.enter_context(tc.tile_pool(name=f"ln_row{suffix}", bufs=2))
                tmpp = ctx.enter_context(tc.tile_pool(name=f"ln_tmp{suffix}", bufs=3))
                for nt in range(NNT):
                    cs = slice(nt * NT, (nt + 1) * NT)
                    st0 = stp.tile([1, NT], F32, tag="st0")
                    for ct in range(2):
                        nc.tensor.matmul(st0[:], onescol_b[:], xrb[ct][:, cs],
                                         start=(ct == 0), stop=(ct == 1))
                    st1 = stp.tile([1, NT], F32, tag="st1")
                    for ct in range(2):
                        sq = sqp.tile([128, NT], BF)
                        with nc.allow_low_precision("bf16 LN square"):
                            nc.vector.tensor_mul(sq[:], xrb[ct][:, cs],
                                                 xrb[ct][:, cs])
                        nc.tensor.matmul(st1[:], onescol_b[:], sq[:],
                                         start=(ct == 0), stop=(ct == 1))
                    m2 = rowp.tile([1, NT], F32, tag="m2")
                    nc.scalar.activation(m2[:], st0[:], Act.Square, scale=1.0 / C)
                    var = rowp.tile([1, NT], F32, tag="var")
                    nc.vector.scalar_tensor_tensor(var[:], st1[:], 1.0 / C, m2[:],
                                                   op0=Alu.mult, op1=Alu.subtract)
                    # rstd = exp(-0.5*ln(var+eps)): ln+exp share an ACT table
                    # set (sqrt does not), so LN never forces a table reload
                    # against the attention exps; also frees DVE (no recip).
                    lv = rowp.tile([1, NT], F32, tag="lv")
                    nc.scalar.activation(lv[:], var[:], Act.Ln, bias=epscol[0:1, :])
                    arow = rowp.tile([1, NT], BF, tag="arow")
                    with nc.allow_low_precision("bf16 rstd broadcast row"):
                        nc.scalar.activation(arow[:], lv[:], Act.Exp, scale=-0.5)
                    crow = rowp.tile([1, NT], BF, tag="crow")
                    nc.vector.scalar_tensor_tensor(crow[:], st0[:], -1.0 / C, arow[:],
                                                   op0=Alu.mult, op1=Alu.mult)
                    aps = bcp.tile([128, NT], F32, tag="abc")
                    nc.tensor.matmul(aps[:], onesr[:], arow[:])
                    cps = bcp.tile([128, NT], F32, tag="abc")
                    nc.tensor.matmul(cps[:], onesr[:], crow[:])
                    asb = tmpp.tile([128, NT], BF, tag="asb")
                    csb = tmpp.tile([128, NT], BF, tag="csb")
                    if act_evict:
                        nc.scalar.copy(asb[:], aps[:])
                        nc.scalar.copy(csb[:], cps[:])
                    else:
                        nc.vector.tensor_copy(asb[:], aps[:])
                        nc.vector.tensor_copy(csb[:], cps[:])
                    with nc.allow_low_precision("low-precision LN apply"):
                        for ct in range(2):
                            t0 = tmpp.tile([128, NT], BF, tag="t0")
                            nc.vector.tensor_mul(t0[:], xrb[ct][:, cs], asb[:])
                            nc.vector.tensor_add(dst(ct, cs), t0[:], csb[:])
                    if hook is not None:
                        hook(nt)

        # ================= stage 1: LN1 + attention =================
        def run_stage1(it):
            ctx = ExitStack()
            apool = ctx.enter_context(tc.tile_pool(name="attn_sb", bufs=1))
            cT8 = apool.tile([128, 2, KV], F8, tag="cT8", name="cT8")
            k_sb = [apool.tile([128, KV], BF, tag=f"k{t}", name=f"k{t}") for t in range(2)]
            v_sb = apool.tile([128, NKT * 260], BF, tag="v", name="v_sb")
            q_sb = [apool.tile([128, N], BF, tag=f"q{t}", name=f"q{t}") for t in range(2)]
            oc8 = apool.tile([128, 2, N], F8, tag="oc8", name="oc8")

            with ExitStack() as pctx:
                mmp = pctx.enter_context(tc.tile_pool(name="proj_ps", bufs=2, space="PSUM"))

                def ln1_hook(nt):
                    # SR 2x2/s2 depthwise conv rows + q projection for this
                    # tile: pipelines with the remaining LN1 tiles.
                    cs = slice(nt * NT, (nt + 1) * NT)
                    r = slice(nt * 4, (nt + 1) * 4)
                    with nc.allow_low_precision("fp8 SR conv"):
                        for ct in range(2):
                            h4 = hb8[:, ct, :].rearrange("p (h a w b) -> p h a w b",
                                                         a=2, b=2, h=28, w=28)
                            c3 = cT8[:, ct, :].rearrange("p (h w) -> p h w", w=28)
                            nc.vector.tensor_scalar_mul(c3[:, r, :], h4[:, r, 0, :, 0],
                                                        srw_sb[ct][:, 0:1])
                            for ky, kx in ((0, 1), (1, 0), (1, 1)):
                                ti = ky * 2 + kx
                                nc.vector.scalar_tensor_tensor(
                                    c3[:, r, :], h4[:, r, ky, :, kx], srw_sb[ct][:, ti:ti + 1],
                                    c3[:, r, :], op0=Alu.mult, op1=Alu.add)
                    for mt in range(2):
                        ps = mmp.tile([128, NT], F32, tag="mm")
                        nc.tensor.matmul(ps[:],
                                         wq_sb[:, :, mt * 128:(mt + 1) * 128],
                                         hb8[:, :, cs], perf_mode=DR,
                                         start=True, stop=True)
                        nc.scalar.activation(q_sb[mt][:, cs], ps[:], Act.Identity,
                                             bias=bq_sb[:, mt:mt + 1])

                layer_norm("1" + it, lambda ct, cs: hb8[:, ct, cs],
                           hook=ln1_hook, act_evict=True)

                # k^T = wk @ cT -> [256, 784] bf16 (no bias: bk cancels in softmax)
                for mt in range(2):
                    for n0, nsz in ((0, 448), (448, 336)):
                        ps = mmp.tile([128, NT], F32, tag="mm")
                        nc.tensor.matmul(ps[:, :nsz],
                                         wk_sb[:, :, mt * 128:(mt + 1) * 128],
                                         cT8[:, :, n0:n0 + nsz],
                                         perf_mode=DR, start=True, stop=True)
                        nc.vector.tensor_copy(k_sb[mt][:, n0:n0 + nsz], ps[:, :nsz])

                # v (+ones col per head) -> v_sb [112, 7*260] bf16 (no bias:
                # bv folds into bo)
                for kt in range(NKT):
                    ps = mmp.tile([128, NT], F32, tag="mm")
                    nc.tensor.matmul(ps[0:KT, 0:C], cT8[:, :, kt * KT:(kt + 1) * KT],
                                     wv_sb[:], perf_mode=DR, start=True, stop=True)
                    v4 = v_sb[0:KT, kt * 260:(kt + 1) * 260].rearrange(
                        "p (h e) -> p h e", e=65)
                    nc.vector.tensor_copy(
                        v4[:, :, 0:64],
                        ps[0:KT, 0:C].rearrange("p (h e) -> p h e", e=64))
                    nc.vector.memset(v4[:, :, 64:65], 1.0)

            # flash attention over head-pairs: per (qt, ht) the two heads'
            # K=64 score matmuls go to PE row-groups 0-63 / 64-127 (derived
            # from lhsT base_partition) and different PSUM banks, so they run
            # concurrently. exp covers both heads in one ACT op; rel-pos bias
            # applied as exp(s)*exp(rp) with host-precomputed exp(rp) on DVE.
            with ExitStack() as pctx:
                rpp = pctx.enter_context(tc.tile_pool(name="rp", bufs=4))
                ppp = pctx.enter_context(tc.tile_pool(name="pexp", bufs=2))
                etp = pctx.enter_context(tc.tile_pool(name="et", bufs=4))
                sps = pctx.enter_context(tc.tile_pool(name="spsum", bufs=2, space="PSUM"))
                ops = pctx.enter_context(tc.tile_pool(name="opsum", bufs=2, space="PSUM"))
                rps = pctx.enter_context(tc.tile_pool(name="rpsum", bufs=1, space="PSUM"))
                wop = pctx.enter_context(tc.tile_pool(name="wo_ps", bufs=1, space="PSUM"))
                rsp = pctx.enter_context(tc.tile_pool(name="rsb", bufs=2))

                def wo_qt(qt):
                    # wo projection + residual for this q-tile, emitted as
                    # soon as both head-pairs' AV is done: spreads the DVE
                    # residual adds across the attention phase.
                    cs = slice(qt * NT, (qt + 1) * NT)
                    for mt in range(2):
                        ps = wop.tile([128, NT], F32, name="wops")
                        nc.tensor.matmul(ps[:], wo_sb[:, :, mt * 128:(mt + 1) * 128],
                                         oc8[:, :, cs], perf_mode=DR,
                                         start=True, stop=True)
                        nc.vector.scalar_tensor_tensor(xres[mt][:, cs], ps[:], bo_sb[:, mt:mt + 1],
                                                       xres[mt][:, cs], op0=Alu.add, op1=Alu.add)
                        nc.gpsimd.tensor_copy(xrb[mt][:, cs], xres[mt][:, cs])

                def do_av(qt, ht, p_t):
                    cs = slice(qt * NT, (qt + 1) * NT)
                    for hh in range(2):
                        h = 2 * ht + hh
                        o_ps = ops.tile([65, NT], F32, name="o_ps")
                        for kt in range(NKT):
                            nc.tensor.matmul(
                                o_ps[:],
                                v_sb[0:KT, kt * 260 + h * 65: kt * 260 + (h + 1) * 65],
                                p_t[:, kt, hh, :], start=(kt == 0), stop=(kt == NKT - 1))
                        rrow = rsp.tile([1, NT], BF, tag="rrow", name="rrow")
                        with nc.allow_low_precision("bf16 softmax denom row"):
                            nc.vector.reciprocal(rrow[:], o_ps[64:65, :])
                        rb_ps = rps.tile([64, NT], F32, name="rb_ps")
                        nc.tensor.matmul(rb_ps[:], onesr[0:1, 0:64], rrow[:])
                        rb_sb = rsp.tile([64, NT], F32, tag="rbsb", name="rb_sb")
                        nc.vector.tensor_copy(rb_sb[:], rb_ps[:])
                        with nc.allow_low_precision("fp8 attn output"):
                            nc.vector.tensor_mul(
                                oc8[hh * 64:(hh + 1) * 64, ht, cs],
                                o_ps[0:64, :], rb_sb[:])

                lag = None
                for qt in range(NNT):
                    cs = slice(qt * NT, (qt + 1) * NT)
                    for ht in range(2):
                        rp_t = rpp.tile([KT, NKT, 2, NT], BF, name="rp_t")
                        # issue from the Pool DGE queue: SP's queue is busy
                        # serially issuing the input/weight loads at startup,
                        # which otherwise delays the first rp prefetches
                        nc.gpsimd.dma_start(out=rp_t[:], in_=rpT_d.ap()[ht, qt])
                        p_t = ppp.tile([KT, NKT, 2, NT], BF, name="p_t")
                        for kt in range(NKT):
                            s_ps = sps.tile([KT, 1024], F32, name="s_ps")
                            s3 = s_ps[:].rearrange("p (a b) -> p a b", b=512)
                            for hh in range(2):
                                nc.tensor.matmul(
                                    s3[:, hh, 0:NT],
                                    k_sb[ht][hh * 64:(hh + 1) * 64, kt * KT:(kt + 1) * KT],
                                    q_sb[ht][hh * 64:(hh + 1) * 64, cs],
                                    start=True, stop=True)
                            et = etp.tile([KT, 2, NT], BF, name="et")
                            nc.scalar.activation(et[:, :, :], s3[:, :, 0:NT], Act.Exp)
                            eng = nc.gpsimd if kt <= 1 else nc.vector
                            eng.tensor_mul(p_t[:, kt, :, :], et[:, :, :],
                                           rp_t[:, kt, :, :])
                        if lag is not None:
                            do_av(*lag)
                            if lag[1] == 1:
                                wo_qt(lag[0])
                        lag = (qt, ht, p_t)
                do_av(*lag)
                wo_qt(lag[0])

            ctx.close()

        # ================= stage 2: LN2 + conv-MLP + blk dwconv =================
        # dwconv inputs are halo-padded to 58x58 so all 9 taps are always
        # full-window; the MLP dwconv pads with -c1/a1 (so the folded bn1
        # shift c1 sees an effective zero), the blk dwconv pads with zero.

        def run_stage2(it):
            ctx = ExitStack()
            layer_norm("2" + it, lambda ct, cs: hbuf[ct][:, cs], act_evict=True)
            mpool = ctx.enter_context(tc.tile_pool(name="mlp_ps", bufs=3, space="PSUM"))
            dps = ctx.enter_context(tc.tile_pool(name="dw_ps", bufs=2, space="PSUM"))
            upool = ctx.enter_context(tc.tile_pool(name="u", bufs=3))
            accp = ctx.enter_context(tc.tile_pool(name="dwacc", bufs=2))
            digp = ctx.enter_context(tc.tile_pool(name="diag", bufs=2))
            y2p = ctx.enter_context(tc.tile_pool(name="y2", bufs=1))
            y28 = [y2p.tile([128, 2, N], F8, tag=f"y28_{j}", name=f"y28_{j}")
                   for j in range(4)]
            y2 = [y28[m // 2][:, m % 2, :] for m in range(8)]
            x3p = [y2p.tile([128, WP * WP], F32, tag=f"x3p{t}", name=f"x3p{t}")
                   for t in range(2)]
            x3b = [y2p.tile([128, WP * WP], BF, tag=f"x3b{t}", name=f"x3b{t}")
                   for t in range(2)]

            def build_diag(w9_sb):
                diag = []
                for t in range(9):
                    dg = digp.tile([128, 128], BF, tag=f"dg{t}", name=f"dg{t}")
                    nc.vector.tensor_scalar_mul(dg[:], eyeb[:], w9_sb[:, t:t + 1])
                    diag.append(dg)
                return diag

            def dw_pe(src3, diag, bias_col, dst):
                """3x3 depthwise conv of halo-padded bf16 src3 [128,58,58] via
                PE diag matmuls; gelu evict with bias -> dst bf16."""
                for nt in range(NNT):
                    ps = dps.tile([128, NT], F32, name="dwps")
                    r0 = nt * 8
                    for ti, (dy, dx) in enumerate(TAPS9):
                        nc.tensor.matmul(
                            ps[:], diag[ti][:],
                            src3[:, r0 + 1 + dy:r0 + 9 + dy, 1 + dx:57 + dx],
                            start=(ti == 0), stop=(ti == 8))
                    nc.scalar.activation(dst[:, r0 * WS:(r0 + 8) * WS], ps[:], Act.Gelu,
                                         bias=bias_col)

            def dw_elem(src3, w9_sb, bias_col, dst, splits):
                """3x3 depthwise conv on elementwise engines (bf16
                accumulator), row-split per `splits` = [(eng, rlo, rhi)]:
                center-tap init (+bias), 8 fused taps, gelu evicts on ACT."""
                acc = accp.tile([128, N], BF, name="acc")
                a3 = acc[:].rearrange("p (h w) -> p h w", w=WS)
                with nc.allow_low_precision("bf16 dwconv accumulator"):
                    for eng, rlo, rhi in splits:
                        eng.tensor_scalar(a3[:, rlo:rhi, :],
                                          src3[:, rlo + 1:rhi + 1, 1:57], w9_sb[:, 4:5],
                                          bias_col, op0=Alu.mult, op1=Alu.add)
                        for dy, dx in TAPS9:
                            if (dy, dx) == (0, 0):
                                continue
                            t = tap_idx(dy, dx)
                            eng.scalar_tensor_tensor(
                                a3[:, rlo:rhi, :],
                                src3[:, rlo + 1 + dy:rhi + 1 + dy, 1 + dx:57 + dx],
                                w9_sb[:, t:t + 1], a3[:, rlo:rhi, :],
                                op0=Alu.mult, op1=Alu.add)
                for _, rlo, rhi in splits:
                    nc.scalar.activation(dst[:, rlo * WS:rhi * WS],
                                         acc[:, rlo * WS:rhi * WS], Act.Gelu)

            for m in range(8):
                u = upool.tile([128, WP * WP], BF, name="u")
                u3 = u[:].rearrange("p (h w) -> p h w", w=WP)
                pr_c = prow[m][:].rearrange("p (h w) -> p h w", w=1)
                pr_r = prow[m][:].rearrange("p (h w) -> p h w", h=1)
                nc.gpsimd.tensor_copy(u3[:, :, 0:1], pr_c)
                nc.gpsimd.tensor_copy(u3[:, :, 57:58], pr_c)
                nc.gpsimd.tensor_copy(u3[:, 0:1, :], pr_r)
                nc.gpsimd.tensor_copy(u3[:, 57:58, :], pr_r)
                dw_acc = None
                if m == 0:
                    dw_acc = accp.tile([128, N], BF, name="acc")
                    da3 = dw_acc[:].rearrange("p (h w) -> p h w", w=WS)

                def dw_chunk(j):
                    # DVE 3x3 dwconv for one 8-row block: needs u3 rows
                    # j*8..j*8+10 => evicts j and j+1 done (lag-1 behind w1)
                    r0 = j * 8
                    w9 = dw9_sb[m]
                    with nc.allow_low_precision("bf16 dwconv accumulator"):
                        nc.vector.tensor_scalar(
                            da3[:, r0:r0 + 8, :], u3[:, r0 + 1:r0 + 9, 1:57],
                            w9[:, 4:5], dwb_sb[:, m:m + 1], op0=Alu.mult, op1=Alu.add)
                        for dy, dx in TAPS9:
                            if (dy, dx) == (0, 0):
                                continue
                            t = tap_idx(dy, dx)
                            nc.vector.scalar_tensor_tensor(
                                da3[:, r0:r0 + 8, :],
                                u3[:, r0 + 1 + dy:r0 + 9 + dy, 1 + dx:57 + dx],
                                w9[:, t:t + 1], da3[:, r0:r0 + 8, :],
                                op0=Alu.mult, op1=Alu.add)
                    nc.scalar.activation(y2[m][:, r0 * WS:(r0 + 8) * WS],
                                         dw_acc[:, r0 * WS:(r0 + 8) * WS], Act.Gelu)

                for nt in range(NNT):
                    cs = slice(nt * NT, (nt + 1) * NT)
                    ps = mpool.tile([128, NT], F32, tag="mm", name="mmps")
                    for kt in range(2):
                        nc.tensor.matmul(ps[:], w1_sb[kt][:, m * 128:(m + 1) * 128],
                                         hbuf[kt][:, cs], start=(kt == 0), stop=(kt == 1))
                    nc.scalar.activation(u3[:, nt * 8 + 1:(nt + 1) * 8 + 1, 1:57], ps[:],
                                         Act.Gelu, bias=b1_sb[:, m:m + 1])
                    if dw_acc is not None and nt >= 1:
                        dw_chunk(nt - 1)
                if dw_acc is not None:
                    dw_chunk(NNT - 1)
                else:
                    diag = build_diag(dw9_sb[m])
                    dw_pe(u3, diag, dwb_sb[:, m:m + 1], y2[m])

            # w2 (+bn2/pbn folded bias) + residual -> x3p (padded, f32) + bf16
            # copy, with the final blk dwconv fused in at lag-1 row-blocks:
            # blk(j) needs x3 rows j*8..j*8+8 => ready after w2(j+1). ct=0's
            # neighbor taps accumulate in bf16 on DVE, ct=1 runs on PE via
            # diag matmuls + bias ones-row matmul; both fuse the exact-fp32
            # center/residual in the evict: f = acc + (1 + w_center) * x3.
            taps8 = [t for t in TAPS9 if t != (0, 0)]
            blkdiag = [build_diag(bk9_sb[0]), build_diag(bk9_sb[1])]
            x3v = [(x3p[ct][:].rearrange("p (h w) -> p h w", w=WP),
                    x3b[ct][:].rearrange("p (h w) -> p h w", w=WP)) for ct in range(2)]
            for ct in range(2):
                for t3 in x3v[ct]:
                    nc.vector.memset(t3[:, :, 0:1], 0.0)
                    nc.vector.memset(t3[:, :, 57:58], 0.0)
                    nc.vector.memset(t3[:, 0:1, :], 0.0)
                    nc.vector.memset(t3[:, 57:58, :], 0.0)

            def blk_nt(nt):
                # both ct tiles on PE (diag matmuls; the tail has PE slack
                # and PSUM f32 accumulation beats the old bf16 DVE taps)
                r0 = nt * 8
                for ct in range(2):
                    xp3, xb3 = x3v[ct]
                    ps = dps.tile([128, NT], F32, name="blkps")
                    nc.tensor.matmul(ps[:], bkb_row[0:1, ct * 128:(ct + 1) * 128],
                                     onesn[:], start=True, stop=False)
                    for ti, (dy, dx) in enumerate(taps8):
                        nc.tensor.matmul(
                            ps[:], blkdiag[ct][tap_idx(dy, dx)][:],
                            xb3[:, r0 + 1 + dy:r0 + 9 + dy, 1 + dx:57 + dx],
                            start=False, stop=(ti == len(taps8) - 1))
                    fo = accp.tile([128, NT], F32, tag="fout", name="fout", bufs=3)
                    f3 = fo[:].rearrange("p (h w) -> p h w", w=WS)
                    nc.vector.scalar_tensor_tensor(
                        f3[:, :, :], xp3[:, r0 + 1:r0 + 9, 1:57], bk9_sb[ct][:, 4:5],
                        ps[:].rearrange("p (h w) -> p h w", w=WS),
                        op0=Alu.mult, op1=Alu.add)
                    nc.sync.dma_start(
                        out=fT_d[ct * 128:(ct + 1) * 128, r0 * WS:(r0 + 8) * WS],
                        in_=fo[:])

            for nt in range(NNT):
                for mt in range(2):
                    xp3, xb3 = x3v[mt]
                    cs = slice(nt * NT, (nt + 1) * NT)
                    ps = mpool.tile([128, NT], F32, tag="mm", name="mmps2")
                    for j in range(4):
                        nc.tensor.matmul(ps[:], w2_sb[j][:, :, mt * 128:(mt + 1) * 128],
                                         y28[j][:, :, cs], perf_mode=DR,
                                         start=(j == 0), stop=(j == 3))
                    nc.vector.scalar_tensor_tensor(
                        xp3[:, nt * 8 + 1:(nt + 1) * 8 + 1, 1:57], ps[:], b2r_sb[:, mt:mt + 1],
                        xres[mt][:, cs], op0=Alu.add, op1=Alu.add)
                    nc.gpsimd.tensor_copy(xb3[:, nt * 8 + 1:(nt + 1) * 8 + 1, 1:57],
                                          xp3[:, nt * 8 + 1:(nt + 1) * 8 + 1, 1:57])
                if nt >= 1:
                    blk_nt(nt - 1)
            blk_nt(NNT - 1)
            ctx.close()

        for it in range(iters):
            body(f"_i{it}")

    nc.compile()
    bacc.get_activation_tables = orig_tables
    return nc


_CACHE = {}


def _get_program():
    if "nc" not in _CACHE:
        _CACHE["nc"] = _build_program()
    return _CACHE["nc"]


def _prep_inputs(inputs):
    f64 = np.float64
    g1 = inputs["ln1_g"].astype(f64); b1ln = inputs["ln1_b"].astype(f64)
    g2 = inputs["ln2_g"].astype(f64); b2ln = inputs["ln2_b"].astype(f64)
    scale = DH ** -0.5

    def bn_ac(g, b, m, v):
        a = np.asarray(g, f64) / np.sqrt(np.asarray(v, f64) + EPS)
        return a, np.asarray(b, f64) - np.asarray(m, f64) * a

    wq = np.asarray(inputs["wq"], f64); wk = np.asarray(inputs["wk"], f64)
    wv = np.asarray(inputs["wv"], f64); wo = np.asarray(inputs["wo"], f64)

    wq_eff = wq * g1[None, :] * scale
    bq_eff = (wq @ b1ln + np.asarray(inputs["bq"], f64)) * scale

    sa, sc = bn_ac(inputs["srbn_g"], inputs["srbn_b"], inputs["srbn_m"], inputs["srbn_v"])
    srw4 = np.asarray(inputs["sr_w"], f64).reshape(C, 4)  # [c, ky*2+kx]
    srw_eff = srw4 * (g1 * sa)[:, None]
    d_const = sa * (b1ln * srw4.sum(1) + np.asarray(inputs["sr_b"], f64)) + sc
    # bk would add a per-query-row constant to the scores -> cancels in
    # softmax, so k gets no bias at all. bv shifts o by bv (softmax weights
    # sum to 1) -> fold wo @ bv into bo.
    bv_eff = wv @ d_const + np.asarray(inputs["bv"], f64)
    bo_eff = np.asarray(inputs["bo"], f64) + wo @ bv_eff

    w1 = np.asarray(inputs["w1"], f64)
    w1_eff = w1 * g2[None, :]
    b1_eff = w1 @ b2ln + np.asarray(inputs["b1"], f64)
    a1_, c1_ = bn_ac(inputs["bn1_g"], inputs["bn1_b"], inputs["bn1_m"], inputs["bn1_v"])

    # fold bn1 (u' = a1*g + c1) into the dwconv weights: with w' = dw + I_c,
    # out = sum_t w'[t]*u'(+d) + dwb = sum_t (w'[t]*a1)*g(+d) + c1*sum_t w'[t]
    # + dwb, provided g is halo-padded with -c1/a1 (so u'_pad = 0).
    dw9p = np.asarray(inputs["dw_w"], f64).reshape(HID, 9).copy()
    dw9p[:, 4] += 1.0  # residual fold
    dw9_eff = dw9p * a1_[:, None]
    dwb_eff = np.asarray(inputs["dw_b"], f64) + c1_ * dw9p.sum(1)
    padv = -c1_ / a1_

    pa, pc = bn_ac(inputs["pbn_g"], inputs["pbn_b"], inputs["pbn_m"], inputs["pbn_v"])
    a2_, c2_ = bn_ac(inputs["bn2_g"], inputs["bn2_b"], inputs["bn2_m"], inputs["bn2_v"])
    w2 = np.asarray(inputs["w2"], f64)
    w2_eff = (w2 * pa[None, :]) * a2_[:, None]
    b2_eff = a2_ * (w2 @ pc + np.asarray(inputs["b2"], f64)) + c2_

    bk9 = np.asarray(inputs["blkdw_w"], f64).reshape(C, 9).copy()
    bk9[:, 4] += 1.0
    bkb = np.asarray(inputs["blkdw_b"], f64)

    bf = lambda a: np.ascontiguousarray(np.asarray(a, np.float32)).astype(BF16)
    f32 = lambda a: np.ascontiguousarray(np.asarray(a, np.float32))
    E4 = ml_dtypes.float8_e4m3
    f8 = lambda a: np.ascontiguousarray(np.asarray(a, np.float32)).astype(E4)

    def dr2(wT):  # [K=256, M] -> DoubleRow [128, 2, M] fp8
        wT = np.asarray(wT)
        return f8(wT.reshape(2, 128, -1).transpose(1, 0, 2))

    def dr8(wT):  # [K=1024, M] -> 4 x DoubleRow [4, 128, 2, M] fp8
        wT = np.asarray(wT)
        return f8(wT.reshape(4, 2, 128, -1).transpose(0, 2, 1, 3))

    # rp[h, n, m] -> [ht, qt, m_in_tile, kt, h_in_pair, n_in_tile]
    rp6 = np.exp(np.asarray(inputs["relative_pos"], np.float64)).reshape(
        2, 2, NNT, NT, NKT, KT).transpose(0, 2, 5, 4, 1, 3)

    shared = {
        "rpT": np.ascontiguousarray(rp6).astype(BF16),
        "wqT": dr2(wq_eff.T), "wkT": dr2(wk.T), "wvT": dr2(wv.T),
        "woT": dr2(wo.T), "w1T": bf(w1_eff.T), "w2T": dr8(w2_eff.T),
        "bq": f32(bq_eff), "bo": f32(bo_eff), "b1": f32(b1_eff),
        "b2r": f32(b2_eff), "srw": f32(srw_eff), "dw9": f32(dw9_eff),
        "dwb": f32(dwb_eff), "padv": f32(padv),
        "bk9": f32(bk9), "bkb": bf(bkb[None, :]), "bkbc": f32(bkb),
        "eyeb": np.eye(128, dtype=np.float32).astype(BF16),
        "onesr": np.ones((1, 128), np.float32).astype(BF16),
    }
    x = np.asarray(inputs["x"], np.float32)
    in_maps = []
    for b in range(B):
        m = dict(shared)
        xt = np.ascontiguousarray(x[b].T)
        m["xT"] = xt
        m["xTb"] = xt.astype(BF16)
        in_maps.append(m)
    return in_maps


def kernel(**inputs):
    from concourse.bass_utils import run_bass_kernel_spmd
    nc = _get_program()
    in_maps = _prep_inputs(inputs)
    res = run_bass_kernel_spmd(nc, in_maps, core_ids=list(range(B)))
    out = np.stack([res.results[b]["fT"].T for b in range(B)], axis=0)
    return np.ascontiguousarray(out, dtype=np.float32)
